# revision 23
# baseline (speedup 1.0000x reference)
"""Multi-head cross-batch attention (B=4096, d_model=512, H=8 heads) on 8 TRN2 cores.

Sharding: one head per NeuronCore (tensor-parallel over H). Each core computes
its head's Q/K/V projections from a replicated (pre-transposed) x, the full
[4096, 4096] score block for that head, softmax (transposed layout, denominator
via a ones-column in V), attn @ V, and its partial out-projection
Y_h = attn_h @ Wo[:, h*64:(h+1)*64].T. Host sums the 8 partials and adds bo.

Layout notes (per core):
  - xT [512, 4096] (c on partitions) is fed from host so every matmul can
    contract over the partition dim without any on-device transpose of x.
  - QT/KT are stored duplicated across partition halves ([128, 4096]) so score
    matmuls can be row-packed two-at-a-time into the 128x128 PE array (the
    contraction dim is only 64).
  - V is projected in V^T form ([64, tokens], stream-bound: N=512 matmuls),
    padded to 80 rows whose row 64 is ones, and moved to token-major vp layout
    by per-128-token DMA XBAR transposes on the sync HWDGE queue (the XBAR
    ucode mishandles multi-col-block inputs, verified), keeping the PE out of
    the layout change entirely and embedding the ones column at offset 64.
  - Scores are computed transposed (ST[j, m]) so softmax's sum over keys j can
    ride the attn@V matmul: V is augmented with a ones column, making the
    accumulated output row 64 equal to sum_j exp(s). No max-subtraction is
    needed: scores are O(1) here (verified), so exp cannot overflow.
  - Normalization commutes with the out-projection and is applied to the bf16
    numerator along its free dim: fp32 reciprocal of the r row, a single K=1
    broadcast matmul to replicate it across 64 partitions, one multiply.
  - Matmul inputs are bf16 (1 PE pass vs 2 for fp32, fast weight load);
    accumulation is fp32 in PSUM, exp inputs and the softmax denominator stay
    fp32. y partials return in bf16 (host accumulates in fp32).
  - A short warm-up matmul stream runs while x streams in, so the PE's HAM
    clock gate reaches 8/8 before the first projection matmul.
  - Emission is software-pipelined: attnV trails its scores/exp by one group so
    the PE's in-order queue never blocks on an in-flight exp; each chunk's
    output phase is emitted mid-way through the next chunk; chunk 0's score
    groups are interleaved into the projection loop as x arrives n-major;
    chunk n's Q projection is deferred into chunk n-1's group stream.
"""

import sys

if "/opt/trn_rl_repo" not in sys.path:
    sys.path.insert(0, "/opt/trn_rl_repo")

import ml_dtypes
import numpy as np

import concourse.bass as bass
import concourse.tile as tile
from concourse import bacc, mybir

B = 4096
D = 512
H = 8
DK = 64
MC = 512  # query-chunk (m) width
N_MC = B // MC  # 8
JB = B // 128  # 32 j-blocks of 128 keys
F32 = mybir.dt.float32
BF16 = mybir.dt.bfloat16
MM_DT = BF16
NP_MM_DT = ml_dtypes.bfloat16 if MM_DT == BF16 else np.float32

# packed weight layout: per c-chunk of 128 channels, [wq_dup(128) | wk_dup(128) | wv(64)]
WCOLS = 128 + 128 + DK  # 320

# j-blocks per score/exp group: 3 blocks = 1536 floats = 3 PSUM banks.
# PSUM budget: 2x3 (score staging) + 1 (attnV accum) + 1 (vproj/outproj) = 8.
JGROUPS = [(0, 3), (3, 3), (6, 3), (9, 3), (12, 3), (15, 3), (18, 3), (21, 3), (24, 3), (27, 3), (30, 2)]

_NC_CACHE = None


def build_nc():
    nc = bacc.Bacc()

    xt = nc.dram_tensor("xt", [D, B], MM_DT, kind="ExternalInput")
    w3 = nc.dram_tensor("w3", [D, WCOLS], MM_DT, kind="ExternalInput")  # [c, wq|wk|wv]
    b3 = nc.dram_tensor("b3", [128, 3], F32, kind="ExternalInput")  # bq_dup | bk_dup | bv(64, padded)
    wot = nc.dram_tensor("wot", [DK, D], MM_DT, kind="ExternalInput")
    y = nc.dram_tensor("y", [B, D], MM_DT, kind="ExternalOutput")

    with tile.TileContext(nc) as tc:
        with (
            tc.tile_pool(name="const", bufs=1) as const,
            tc.tile_pool(name="epool", bufs=20) as epool,
            tc.tile_pool(name="vtpool", bufs=2) as vtpool,
            tc.tile_pool(name="otpool", bufs=3) as otpool,
            tc.tile_pool(name="ypool", bufs=3) as ypool,
            tc.tile_pool(name="score_ps", bufs=2, space="PSUM") as score_ps,
            tc.tile_pool(name="attnv_ps", bufs=1, space="PSUM") as attnv_ps,
            tc.tile_pool(name="out_ps", bufs=1, space="PSUM") as out_ps,
        ):
            # ---- persistent SBUF ----
            x_sb = const.tile([128, 4 * B], MM_DT)  # 4 c-chunks side by side
            w3_sb = const.tile([128, 4 * WCOLS], MM_DT)  # 4 c-chunks of [128,320]
            b3_sb = const.tile([128, 3], F32)
            wot_sb = const.tile([DK, D], MM_DT)
            warm_sb = const.tile([128, MC], MM_DT)
            ones65 = const.tile([65, DK], MM_DT)  # row 64 = ones (K=1 broadcast matmul lhsT)
            qt_sb = const.tile([128, B], MM_DT)  # QT dup'd across partition halves
            kt_sb = const.tile([128, B], MM_DT)
            vp_sb = const.tile([128, JB * 80], MM_DT)  # [V(64) | 1 | pad(15)] per j-block

            # ---- input DMAs ----
            # x chunk 0 first, split across both HWDGE queues; packed weights
            # next on sync; remaining x chunks round-robin over the queues.
            x_sb3 = x_sb[:].rearrange("p (c n) -> p c n", c=4)
            xt3 = xt[:].rearrange("(c p) n -> p c n", p=128)
            nc.sync.dma_start(out=x_sb3[:, 0:2, 0:MC], in_=xt3[:, 0:2, 0:MC])
            nc.scalar.dma_start(out=x_sb3[:, 2:4, 0:MC], in_=xt3[:, 2:4, 0:MC])
            nc.sync.dma_start(
                out=w3_sb[:].rearrange("p (c n) -> p c n", c=4),
                in_=w3[:].rearrange("(c p) n -> p c n", p=128),
            )
            nc.sync.dma_start(out=b3_sb[:], in_=b3[:])
            nc.sync.dma_start(out=wot_sb[:], in_=wot[:])
            x_dma_eng = {1: nc.gpsimd, 2: nc.sync, 3: nc.scalar, 4: nc.gpsimd, 5: nc.sync, 6: nc.scalar, 7: nc.gpsimd}
            for n in range(1, N_MC):
                x_dma_eng[n].dma_start(
                    out=x_sb3[:, :, n * MC : (n + 1) * MC],
                    in_=xt3[:, :, n * MC : (n + 1) * MC],
                )
            nc.vector.memset(warm_sb[:], 0.125)
            nc.vector.memset(ones65[DK : DK + 1, :], 1.0)
            # preload the exp activation table set before the first real exp
            # (junk output target so nothing downstream depends on it)
            junk_sb = const.tile([65, 4], F32)
            nc.scalar.activation(
                junk_sb[DK : DK + 1, :], ones65[DK : DK + 1, 0:4], mybir.ActivationFunctionType.Exp, scale=0.125
            )

            # ---- PE warm-up: long-stream matmuls (high busy duty) while x
            # streams in, so the HAM clock gate is at 8/8 when the first
            # projection matmul runs.
            for _ in range(10):
                wp = out_ps.tile([DK, MC], F32, tag="out")
                nc.tensor.matmul(wp[:], warm_sb[:, 0:DK], warm_sb[:], start=True, stop=True)

            # ---- emission helpers ----
            def emit_proj(n, off, pool=None, tag="score"):
                # one staging-ring unit holds one projection chunk [128, 512].
                # off 0 -> Q (w3 cols 0:128, bias col 0), off 1 -> K (cols 128:256, bias col 1)
                pp = (pool or score_ps).tile([128, MC], F32, tag=tag)
                for c in range(4):
                    nc.tensor.matmul(
                        pp[:],
                        w3_sb[:, c * WCOLS + off * 128 : c * WCOLS + off * 128 + 128],
                        x_sb[:, c * B + n * MC : c * B + (n + 1) * MC],
                        start=(c == 0),
                        stop=(c == 3),
                    )
                dst = qt_sb if off == 0 else kt_sb
                nc.vector.tensor_scalar(
                    out=dst[:, n * MC : (n + 1) * MC], in0=pp[:],
                    scalar1=b3_sb[:, off : off + 1], scalar2=None, op0=mybir.AluOpType.add,
                )

            def emit_vt(n):
                # V^T chunk [64, 512] (stream-bound), bias via per-partition add
                # into an 80-row padded tile (row 64 = ones), then one XBAR
                # transpose per 128-token block into the token-major vp layout.
                vps = out_ps.tile([DK, MC], F32, tag="out")
                for c in range(4):
                    nc.tensor.matmul(
                        vps[:],
                        w3_sb[:, c * WCOLS + 256 : c * WCOLS + 256 + DK],
                        x_sb[:, c * B + n * MC : c * B + (n + 1) * MC],
                        start=(c == 0),
                        stop=(c == 3),
                    )
                vt = vtpool.tile([80, MC], MM_DT, tag="vt")
                nc.vector.memset(vt[DK:80, :], 1.0)
                nc.vector.tensor_scalar(
                    out=vt[0:DK, :], in0=vps[:],
                    scalar1=b3_sb[0:DK, 2:3], scalar2=None, op0=mybir.AluOpType.add,
                )
                for t in range(4):
                    jb = 4 * n + t
                    nc.sync.dma_start(
                        out=vp_sb[:, jb * 80 : jb * 80 + 80],
                        in_=vt[:, t * 128 : (t + 1) * 128],
                        transpose=True,
                    )

            def emit_sc(mc, g0, gn):
                m0 = mc * MC
                sp = score_ps.tile([128, gn * MC], F32, tag="score")
                et = epool.tile([128, gn * MC], MM_DT, tag="E")
                for k in range(gn):
                    jb = g0 + k
                    h0 = 64 * (jb % 2)
                    nc.tensor.matmul(
                        sp[:, k * MC : (k + 1) * MC],
                        kt_sb[h0 : h0 + 64, jb * 128 : (jb + 1) * 128],
                        qt_sb[h0 : h0 + 64, m0 : m0 + MC],
                        start=True,
                        stop=True,
                    )
                nc.scalar.activation(et[:], sp[:], mybir.ActivationFunctionType.Exp, scale=0.125)
                return et

            def emit_av(mc, g0, gn, et, av):
                for k in range(gn):
                    jb = g0 + k
                    nc.tensor.matmul(
                        av[:],
                        vp_sb[:, jb * 80 : jb * 80 + DK + 1],
                        et[:, k * MC : (k + 1) * MC],
                        start=(jb == 0),
                        stop=(jb == JB - 1),
                    )

            def emit_otcopy(av):
                # Phase 1 of the output path: just free av — bf16 reciprocal of
                # the r row and a copy of the numerator. DVE only, so the PE
                # queue flows straight into the next chunk's score groups.
                rr = otpool.tile([DK + 1, MC], MM_DT, tag="rr")
                with nc.allow_low_precision(reason="softmax denominators are O(1); bf16 1/r costs ~0.2% rel"):
                    nc.vector.reciprocal(rr[DK : DK + 1, :], av[DK : DK + 1, :])
                otb = otpool.tile([DK, MC], MM_DT, tag="ot")
                nc.vector.tensor_copy(otb[:], av[0:DK, :])
                return rr, otb

            def emit_norm(rr, otb):
                # Phase 2 (one group later, reciprocal long done): K=1 bf16
                # matmul broadcasts 1/r across 64 partitions; normalize the
                # numerator along its free (query) dim.
                rb = out_ps.tile([DK, MC], F32, tag="out")
                nc.tensor.matmul(rb[:], ones65[DK : DK + 1, :], rr[DK : DK + 1, :], start=True, stop=True)
                otn = otpool.tile([DK, MC], MM_DT, tag="otn")
                nc.vector.scalar_tensor_tensor(
                    out=otn[:], in0=otb[:], scalar=1.0, in1=rb[:],
                    op0=mybir.AluOpType.mult, op1=mybir.AluOpType.mult,
                )
                return otn

            def emit_output(mc, otn):
                m0 = mc * MC
                ysb = ypool.tile([128, 4 * MC], MM_DT, tag="y")
                for q in range(4):
                    yp = out_ps.tile([128, MC], F32, tag="out")
                    nc.tensor.matmul(yp[:], otn[:, q * 128 : (q + 1) * 128], wot_sb[:], start=True, stop=True)
                    nc.vector.tensor_copy(ysb[:, q * MC : (q + 1) * MC], yp[:])
                    if q == 1:
                        nc.gpsimd.dma_start(
                            out=y[m0 : m0 + 256, :].rearrange("(q p) d -> p q d", p=128),
                            in_=ysb[:, 0 : 2 * MC].rearrange("p (q d) -> p q d", q=2),
                        )
                nc.sync.dma_start(
                    out=y[m0 + 256 : m0 + MC, :].rearrange("(q p) d -> p q d", p=128),
                    in_=ysb[:, 2 * MC : 4 * MC].rearrange("p (q d) -> p q d", q=2),
                )

            # ---- software-pipelined main emission ----
            # attnV for a group is emitted one group behind its scores/exp, so
            # the PE's in-order queue never blocks on an in-flight exp. Each
            # chunk's output phase is emitted mid-way through the next chunk.
            # Chunk n's Q projection is emitted mid-way through chunk n-1.
            state = {"prev": None, "av": None, "norm_pending": None, "out_pending": None}

            def drain_prev():
                if state["prev"] is None:
                    return
                mc, g0, gn, et = state["prev"]
                state["prev"] = None
                if state["av"] is None:
                    state["av"] = attnv_ps.tile([DK + 1, MC], F32, tag="attnv", name="av")
                emit_av(mc, g0, gn, et, state["av"])
                if state["norm_pending"] is not None:
                    pmc, rr, otb = state["norm_pending"]
                    state["norm_pending"] = None
                    state["out_pending"] = (pmc, emit_norm(rr, otb))
                if g0 + gn == JB:  # chunk complete: free av (DVE-only phase)
                    state["norm_pending"] = (mc, *emit_otcopy(state["av"]))
                    state["av"] = None
                elif g0 >= 12 and state["out_pending"] is not None and state["out_pending"][0] == mc - 1:
                    emit_output(*state["out_pending"])
                    state["out_pending"] = None

            def push(mc, g0, gn):
                et = emit_sc(mc, g0, gn)
                drain_prev()
                state["prev"] = (mc, g0, gn, et)

            # projections interleaved with chunk 0 (x arrives n-major).
            # K and V are needed globally and are emitted as x arrives; Q is
            # needed per-chunk: Q0/Q1 up front, Qn during chunk n-1.
            gi = 0
            for n in range(N_MC):
                emit_proj(n, 1)  # K
                if n <= 1:
                    emit_proj(n, 0)  # Q0, Q1
                emit_vt(n)
                while gi < len(JGROUPS) and JGROUPS[gi][0] + JGROUPS[gi][1] <= 4 * n + 4:
                    push(0, *JGROUPS[gi])
                    gi += 1
            while gi < len(JGROUPS):
                push(0, *JGROUPS[gi])
                gi += 1
            for mc in range(1, N_MC):
                for ig, (g0, gn) in enumerate(JGROUPS):
                    push(mc, g0, gn)
                    if ig == 5 and mc + 1 < N_MC:
                        # Q for the next chunk; staged in out_ps so the score
                        # ring's double-buffer parity is undisturbed.
                        emit_proj(mc + 1, 0, pool=out_ps, tag="out")
            drain_prev()
            pmc, rr, otb = state["norm_pending"]
            emit_output(pmc, emit_norm(rr, otb))
    nc.finalize()
    return nc


def _get_nc():
    global _NC_CACHE
    if _NC_CACHE is None:
        _NC_CACHE = build_nc()
    return _NC_CACHE


def make_in_maps(x, Wq, bq, Wk, bk, Wv, bv, Wo, bo):
    xT = np.ascontiguousarray(np.asarray(x, dtype=np.float32).T).astype(NP_MM_DT)
    maps = []
    for h in range(H):
        s = slice(h * DK, (h + 1) * DK)
        wqT = np.asarray(Wq, np.float32)[s, :].T  # [512, 64]
        wkT = np.asarray(Wk, np.float32)[s, :].T
        wvT = np.asarray(Wv, np.float32)[s, :].T
        w3 = np.concatenate([wqT, wqT, wkT, wkT, wvT], axis=1)  # [512, 320]
        b3 = np.zeros((128, 3), np.float32)
        b3[:, 0] = np.tile(np.asarray(bq, np.float32)[s], 2)
        b3[:, 1] = np.tile(np.asarray(bk, np.float32)[s], 2)
        b3[0:DK, 2] = np.asarray(bv, np.float32)[s]
        maps.append(
            {
                "xt": xT,
                "w3": np.ascontiguousarray(w3).astype(NP_MM_DT),
                "b3": b3,
                "wot": np.ascontiguousarray(np.asarray(Wo, np.float32)[:, s].T).astype(NP_MM_DT),
            }
        )
    return maps


def _ensure_ntff_hook_shim():
    # The image's antenv package lacks axon_hooks; bass_utils imports it when
    # tracing is requested (including via the BASS_TRACE env var). Register a
    # ctypes-backed shim so that path works regardless of environment.
    if "antenv.axon_hooks" in sys.modules:
        return
    try:
        import contextlib
        import ctypes
        import types

        mod = types.ModuleType("antenv.axon_hooks")
        _state = {"hook": None}

        def set_axon_ntff_profile_hook(hook):
            _state["hook"] = hook

        def get_axon_ntff_profile_hook():
            if _state["hook"] is None:
                try:
                    lib = ctypes.CDLL("/opt/axon/libaxon_pjrt.so")
                except OSError:
                    return None
                if not hasattr(lib, "axon_start_nrt_profile"):
                    return None
                lib.axon_start_nrt_profile.argtypes = [ctypes.POINTER(ctypes.c_int64), ctypes.c_size_t]
                lib.axon_start_nrt_profile.restype = ctypes.c_int64
                lib.axon_stop_nrt_profile.argtypes = [ctypes.c_char_p]
                lib.axon_stop_nrt_profile.restype = ctypes.c_int64

                @contextlib.contextmanager
                def _hook(output_dir, device_ids):
                    import jax

                    jax.devices()
                    if device_ids:
                        ids = (ctypes.c_int64 * len(device_ids))(*device_ids)
                        rc = lib.axon_start_nrt_profile(ids, len(device_ids))
                    else:
                        rc = lib.axon_start_nrt_profile(None, 0)
                    if rc != 0:
                        raise RuntimeError(f"axon_start_nrt_profile rc={rc}")
                    try:
                        yield
                    finally:
                        n = lib.axon_stop_nrt_profile(str(output_dir).encode())
                        print(f"profile: {n} file(s) written to {output_dir}", file=sys.stderr)

                _state["hook"] = _hook
            return _state["hook"]

        mod.set_axon_ntff_profile_hook = set_axon_ntff_profile_hook
        mod.get_axon_ntff_profile_hook = get_axon_ntff_profile_hook
        sys.modules["antenv.axon_hooks"] = mod
        try:
            import antenv

            antenv.axon_hooks = mod
        except ImportError:
            pass
    except Exception:
        pass


def run(inputs, trace=False, **kw):
    _ensure_ntff_hook_shim()
    from concourse import bass_utils as BU
    from concourse.bass_utils import run_bass_kernel_spmd

    if not getattr(BU.upload_artifacts, "_safe", False):
        _orig_upload = BU.upload_artifacts

        def _safe_upload(tmpdir):
            try:
                return _orig_upload(tmpdir)
            except Exception:
                return f"local:{tmpdir}"

        _safe_upload._safe = True
        BU.upload_artifacts = _safe_upload

    nc = _get_nc()
    in_maps = make_in_maps(**inputs)
    res = run_bass_kernel_spmd(nc, in_maps, list(range(H)), trace=trace, **kw)
    bo = np.asarray(inputs["bo"], np.float32)
    out = np.zeros((B, D), np.float32)
    for c in range(H):
        out += np.asarray(res.results[c]["y"], dtype=np.float32)
    out += bo[None, :]
    return out, res


def kernel(**inputs):
    out, _ = run(inputs, trace=False)
    return out


# revision 37
# speedup vs baseline: 1.3897x; 1.3897x over previous
"""Multi-head cross-batch attention (B=4096, d_model=512, H=8 heads) on 8 TRN2 cores.

Sharding: one head per NeuronCore (tensor-parallel over H). Each core computes
its head's Q/K/V projections from a replicated (pre-transposed) x, the full
[4096, 4096] score block for that head, softmax (transposed layout, denominator
via a ones-column in V), attn @ V, and its partial out-projection
Y_h = attn_h @ Wo[:, h*64:(h+1)*64].T. Host sums the 8 partials and adds bo.

Layout notes (per core):
  - xT [512, 4096] (c on partitions) is fed from host so every matmul can
    contract over the partition dim without any on-device transpose of x.
  - QT/KT are stored duplicated across partition halves ([128, 4096]) so score
    matmuls can be row-packed two-at-a-time into the 128x128 PE array (the
    contraction dim is only 64).
  - V is projected directly token-major (x chunk as the stationary operand).
    The V bias is dropped on device: softmax rows sum to 1, so bv contributes
    the constant row Wo @ bv to y, which the host adds together with bo
    (exact). Similarly the K bias only shifts each softmax row by a constant
    and could be dropped, but it is kept since the add rides a drain copy.
  - Scores are computed transposed (ST[j, m]) so softmax's sum over keys j can
    ride the attn@V matmul: V is augmented with a ones column, making the
    accumulated output row 64 equal to sum_j exp(s). No max-subtraction is
    needed: scores are O(1) here (verified), so exp cannot overflow.
  - Normalization commutes with the out-projection and is applied to the bf16
    numerator along its free dim: fp32 reciprocal of the r row, a single K=1
    broadcast matmul to replicate it across 64 partitions, one multiply.
  - Matmul inputs are bf16 (1 PE pass vs 2 for fp32, fast weight load);
    accumulation is fp32 in PSUM, exp inputs and the softmax denominator stay
    fp32. y partials return in bf16 (host accumulates in fp32).
  - A short warm-up matmul stream runs while x streams in, so the PE's HAM
    clock gate reaches 8/8 before the first projection matmul.
  - Emission is software-pipelined: attnV trails its scores/exp by one group so
    the PE's in-order queue never blocks on an in-flight exp; each chunk's
    output phase is emitted mid-way through the next chunk; chunk 0's score
    groups are interleaved into the projection loop as x arrives n-major;
    chunk n's Q projection is deferred into chunk n-1's group stream.
"""

import sys
from collections import deque

if "/opt/trn_rl_repo" not in sys.path:
    sys.path.insert(0, "/opt/trn_rl_repo")

import ml_dtypes
import numpy as np

import concourse.bass as bass
import concourse.tile as tile
from concourse import bacc, mybir

B = 4096
D = 512
H = 8
DK = 64
MC = 512  # query-chunk (m) width
N_MC = B // MC  # 8
JB = B // 128  # 32 j-blocks of 128 keys
F32 = mybir.dt.float32
BF16 = mybir.dt.bfloat16
MM_DT = BF16
NP_MM_DT = ml_dtypes.bfloat16 if MM_DT == BF16 else np.float32

# packed weight layout: per c-chunk of 128 channels, [wq_dup(128) | wk_dup(128) | wv(64)]
WCOLS = 128 + 128 + DK  # 320

# j-blocks per score/exp group: 3 blocks = 1536 floats = 3 PSUM banks.
# PSUM budget: 2x3 (score staging) + 1 (attnV accum) + 1 (vproj/outproj) = 8.
JGROUPS = [(0, 3), (3, 3), (6, 3), (9, 3), (12, 3), (15, 3), (18, 3), (21, 3), (24, 3), (27, 3), (30, 2)]

_NC_CACHE = None


def build_nc():
    nc = bacc.Bacc()

    xt = nc.dram_tensor("xt", [D, B], MM_DT, kind="ExternalInput")
    w3 = nc.dram_tensor("w3", [D, WCOLS], MM_DT, kind="ExternalInput")  # [c, wq|wk|wv]
    b3 = nc.dram_tensor("b3", [128, 2], F32, kind="ExternalInput")  # bq_dup | bk_dup
    wot = nc.dram_tensor("wot", [DK, D], MM_DT, kind="ExternalInput")
    y = nc.dram_tensor("y", [B, D], MM_DT, kind="ExternalOutput")

    with tile.TileContext(nc) as tc:
        with (
            tc.tile_pool(name="const", bufs=1) as const,
            tc.tile_pool(name="epool", bufs=20) as epool,
            tc.tile_pool(name="otpool", bufs=3) as otpool,
            tc.tile_pool(name="ypool", bufs=3) as ypool,
            tc.tile_pool(name="score_ps", bufs=2, space="PSUM") as score_ps,
            tc.tile_pool(name="attnv_ps", bufs=1, space="PSUM") as attnv_ps,
            tc.tile_pool(name="out_ps", bufs=1, space="PSUM") as out_ps,
        ):
            # ---- persistent SBUF ----
            x_sb = const.tile([128, 4 * B], MM_DT)  # 4 c-chunks side by side
            w3_sb = const.tile([128, 4 * WCOLS], MM_DT)  # 4 c-chunks of [128,320]
            b3_sb = const.tile([128, 2], F32)
            wot_sb = const.tile([DK, D], MM_DT)
            warm_sb = const.tile([128, MC], MM_DT)
            ones65 = const.tile([65, DK], MM_DT)  # row 64 = ones (K=1 broadcast matmul lhsT)
            qt_sb = const.tile([128, B], MM_DT)  # QT dup'd across partition halves
            kt_sb = const.tile([128, B], MM_DT)
            vp_sb = const.tile([128, JB * (DK + 1)], MM_DT)  # [V | 1] per j-block

            # ---- input DMAs ----
            # Co-queued DMAs on one ring complete nearly together, so the
            # weights get the scalar HWDGE ring to themselves (land first),
            # x chunk 0 gets the sync ring, and the rest of x spreads over all
            # three rings roughly in consumption order.
            x_sb3 = x_sb[:].rearrange("p (c n) -> p c n", c=4)
            xt3 = xt[:].rearrange("(c p) n -> p c n", p=128)
            nc.scalar.dma_start(
                out=w3_sb[:].rearrange("p (c n) -> p c n", c=4),
                in_=w3[:].rearrange("(c p) n -> p c n", p=128),
            )
            nc.scalar.dma_start(out=b3_sb[:], in_=b3[:])
            nc.sync.dma_start(out=x_sb3[:, :, 0:MC], in_=xt3[:, :, 0:MC])
            nc.sync.dma_start(out=wot_sb[:], in_=wot[:])
            x_dma_eng = {1: nc.gpsimd, 2: nc.sync, 3: nc.scalar, 4: nc.gpsimd, 5: nc.sync, 6: nc.scalar, 7: nc.gpsimd}
            for n in range(1, N_MC):
                x_dma_eng[n].dma_start(
                    out=x_sb3[:, :, n * MC : (n + 1) * MC],
                    in_=xt3[:, :, n * MC : (n + 1) * MC],
                )
            nc.vector.memset(warm_sb[:], 0.125)
            nc.vector.memset(ones65[DK : DK + 1, :], 1.0)
            # ones columns of the augmented-V layout
            nc.vector.memset(
                vp_sb[:].rearrange("p (t e) -> p t e", e=DK + 1)[:, :, DK : DK + 1], 1.0
            )
            # preload the exp activation table set before the first real exp
            # (junk output target so nothing downstream depends on it)
            junk_sb = const.tile([65, 4], F32)
            nc.scalar.activation(
                junk_sb[DK : DK + 1, :], ones65[DK : DK + 1, 0:4], mybir.ActivationFunctionType.Exp, scale=0.125
            )

            # ---- PE warm-up: long-stream matmuls (high busy duty) while x
            # streams in, so the HAM clock gate is at 8/8 when the first
            # projection matmul runs.
            for _ in range(10):
                wp = out_ps.tile([DK, MC], F32, tag="out")
                nc.tensor.matmul(wp[:], warm_sb[:, 0:DK], warm_sb[:], start=True, stop=True)

            # ---- emission helpers ----
            def emit_proj(n, off, pool=None, tag="score"):
                # one staging-ring unit holds one projection chunk [128, 512].
                # off 0 -> Q (w3 cols 0:128, bias col 0), off 1 -> K (cols 128:256, bias col 1)
                pp = (pool or score_ps).tile([128, MC], F32, tag=tag)
                for c in range(4):
                    nc.tensor.matmul(
                        pp[:],
                        w3_sb[:, c * WCOLS + off * 128 : c * WCOLS + off * 128 + 128],
                        x_sb[:, c * B + n * MC : c * B + (n + 1) * MC],
                        start=(c == 0),
                        stop=(c == 3),
                    )
                dst = qt_sb if off == 0 else kt_sb
                nc.vector.tensor_scalar(
                    out=dst[:, n * MC : (n + 1) * MC], in0=pp[:],
                    scalar1=b3_sb[:, off : off + 1], scalar2=None, op0=mybir.AluOpType.add,
                )

            def emit_v4(n):
                # V for j-blocks 4n..4n+3 token-major (x chunk stationary), no
                # bias (folded into a host-side constant), one strided drain.
                t0 = 4 * n
                vps = out_ps.tile([128, 4 * DK], F32, tag="out")
                for k in range(4):
                    dst = vps[:, k * DK : (k + 1) * DK]
                    for c in range(4):
                        nc.tensor.matmul(
                            dst,
                            x_sb[:, c * B + (t0 + k) * 128 : c * B + (t0 + k + 1) * 128],
                            w3_sb[:, c * WCOLS + 256 : c * WCOLS + 256 + DK],
                            start=(c == 0),
                            stop=(c == 3),
                        )
                nc.vector.tensor_copy(
                    vp_sb[:].rearrange("p (t e) -> p t e", e=DK + 1)[:, t0 : t0 + 4, 0:DK],
                    vps[:].rearrange("p (t e) -> p t e", e=DK),
                )

            def emit_sc(mc, g0, gn):
                m0 = mc * MC
                sp = score_ps.tile([128, gn * MC], F32, tag="score")
                et = epool.tile([128, gn * MC], MM_DT, tag="E")
                for k in range(gn):
                    jb = g0 + k
                    h0 = 64 * (jb % 2)
                    nc.tensor.matmul(
                        sp[:, k * MC : (k + 1) * MC],
                        kt_sb[h0 : h0 + 64, jb * 128 : (jb + 1) * 128],
                        qt_sb[h0 : h0 + 64, m0 : m0 + MC],
                        start=True,
                        stop=True,
                    )
                nc.scalar.activation(et[:], sp[:], mybir.ActivationFunctionType.Exp, scale=0.125)
                return et

            def emit_av(mc, g0, gn, et, av):
                for k in range(gn):
                    jb = g0 + k
                    nc.tensor.matmul(
                        av[:],
                        vp_sb[:, jb * (DK + 1) : (jb + 1) * (DK + 1)],
                        et[:, k * MC : (k + 1) * MC],
                        start=(jb == 0),
                        stop=(jb == JB - 1),
                    )

            def emit_otcopy(av):
                # Phase 1 of the output path: just free av — bf16 reciprocal of
                # the r row and a copy of the numerator. DVE only, so the PE
                # queue flows straight into the next chunk's score groups.
                rr = otpool.tile([DK + 1, MC], MM_DT, tag="rr")
                with nc.allow_low_precision(reason="softmax denominators are O(1); bf16 1/r costs ~0.2% rel"):
                    nc.vector.reciprocal(rr[DK : DK + 1, :], av[DK : DK + 1, :])
                otb = otpool.tile([DK, MC], MM_DT, tag="ot")
                nc.vector.tensor_copy(otb[:], av[0:DK, :])
                return rr, otb

            def emit_norm(rr, otb):
                # Phase 2 (one group later, reciprocal long done): K=1 bf16
                # matmul broadcasts 1/r across 64 partitions; normalize the
                # numerator along its free (query) dim.
                rb = out_ps.tile([DK, MC], F32, tag="out")
                nc.tensor.matmul(rb[:], ones65[DK : DK + 1, :], rr[DK : DK + 1, :], start=True, stop=True)
                otn = otpool.tile([DK, MC], MM_DT, tag="otn")
                nc.vector.scalar_tensor_tensor(
                    out=otn[:], in0=otb[:], scalar=1.0, in1=rb[:],
                    op0=mybir.AluOpType.mult, op1=mybir.AluOpType.mult,
                )
                return otn

            def emit_output(mc, otn):
                m0 = mc * MC
                ysb = ypool.tile([128, 4 * MC], MM_DT, tag="y")
                for q in range(4):
                    yp = out_ps.tile([128, MC], F32, tag="out")
                    nc.tensor.matmul(yp[:], otn[:, q * 128 : (q + 1) * 128], wot_sb[:], start=True, stop=True)
                    nc.vector.tensor_copy(ysb[:, q * MC : (q + 1) * MC], yp[:])
                    if q == 1:
                        nc.gpsimd.dma_start(
                            out=y[m0 : m0 + 256, :].rearrange("(q p) d -> p q d", p=128),
                            in_=ysb[:, 0 : 2 * MC].rearrange("p (q d) -> p q d", q=2),
                        )
                nc.sync.dma_start(
                    out=y[m0 + 256 : m0 + MC, :].rearrange("(q p) d -> p q d", p=128),
                    in_=ysb[:, 2 * MC : 4 * MC].rearrange("p (q d) -> p q d", q=2),
                )

            # ---- software-pipelined main emission ----
            # attnV for a group is emitted one group behind its scores/exp, so
            # the PE's in-order queue never blocks on an in-flight exp. Each
            # chunk's output phase is emitted mid-way through the next chunk.
            # Chunk n's Q projection is emitted mid-way through chunk n-1.
            state = {"q": deque(), "av": None, "norm_pending": None, "out_pending": None}

            def drain_one():
                mc, g0, gn, et = state["q"].popleft()
                if state["av"] is None:
                    state["av"] = attnv_ps.tile([DK + 1, MC], F32, tag="attnv", name="av")
                emit_av(mc, g0, gn, et, state["av"])
                if state["norm_pending"] is not None:
                    pmc, rr, otb = state["norm_pending"]
                    state["norm_pending"] = None
                    state["out_pending"] = (pmc, emit_norm(rr, otb))
                if g0 + gn == JB:  # chunk complete: free av (DVE-only phase)
                    state["norm_pending"] = (mc, *emit_otcopy(state["av"]))
                    state["av"] = None
                elif g0 >= 12 and state["out_pending"] is not None and state["out_pending"][0] == mc - 1:
                    emit_output(*state["out_pending"])
                    state["out_pending"] = None

            def push(mc, g0, gn):
                # attnV trails scores/exp by TRAIL(mc) groups: deep for chunk 0
                # (spills chunk-0 attnV into later chunks' PE slack while exp
                # tracks the incoming x stream), shallow in steady state.
                et = emit_sc(mc, g0, gn)
                state["q"].append((mc, g0, gn, et))
                trail = {0: 6, 1: 4, 2: 2}.get(mc, 1)
                while len(state["q"]) > trail:
                    drain_one()

            # projections interleaved with chunk 0 (x arrives n-major).
            # K and V are needed globally and are emitted as x arrives; Q is
            # needed per-chunk: Q0/Q1 up front, Qn during chunk n-1.
            gi = 0
            for n in range(N_MC):
                emit_proj(n, 1)  # K
                if n <= 1:
                    emit_proj(n, 0)  # Q0, Q1
                emit_v4(n)
                while gi < len(JGROUPS) and JGROUPS[gi][0] + JGROUPS[gi][1] <= 4 * n + 4:
                    push(0, *JGROUPS[gi])
                    gi += 1
            while gi < len(JGROUPS):
                push(0, *JGROUPS[gi])
                gi += 1
            for mc in range(1, N_MC):
                for ig, (g0, gn) in enumerate(JGROUPS):
                    push(mc, g0, gn)
                    if ig == 5 and mc + 1 < N_MC:
                        # Q for the next chunk; staged in out_ps so the score
                        # ring's double-buffer parity is undisturbed.
                        emit_proj(mc + 1, 0, pool=out_ps, tag="out")
            while state["q"]:
                drain_one()
            pmc, rr, otb = state["norm_pending"]
            emit_output(pmc, emit_norm(rr, otb))
    nc.finalize()
    return nc


def _get_nc():
    global _NC_CACHE
    if _NC_CACHE is None:
        _NC_CACHE = build_nc()
    return _NC_CACHE


def make_in_maps(x, Wq, bq, Wk, bk, Wv, bv, Wo, bo):
    xT = np.ascontiguousarray(np.asarray(x, dtype=np.float32).T).astype(NP_MM_DT)
    maps = []
    for h in range(H):
        s = slice(h * DK, (h + 1) * DK)
        wqT = np.asarray(Wq, np.float32)[s, :].T  # [512, 64]
        wkT = np.asarray(Wk, np.float32)[s, :].T
        wvT = np.asarray(Wv, np.float32)[s, :].T
        w3 = np.concatenate([wqT, wqT, wkT, wkT, wvT], axis=1)  # [512, 320]
        b3 = np.zeros((128, 2), np.float32)
        b3[:, 0] = np.tile(np.asarray(bq, np.float32)[s], 2)
        b3[:, 1] = np.tile(np.asarray(bk, np.float32)[s], 2)
        maps.append(
            {
                "xt": xT,
                "w3": np.ascontiguousarray(w3).astype(NP_MM_DT),
                "b3": b3,
                "wot": np.ascontiguousarray(np.asarray(Wo, np.float32)[:, s].T).astype(NP_MM_DT),
            }
        )
    return maps


def _ensure_ntff_hook_shim():
    # The image's antenv package lacks axon_hooks; bass_utils imports it when
    # tracing is requested (including via the BASS_TRACE env var). Register a
    # ctypes-backed shim so that path works regardless of environment.
    if "antenv.axon_hooks" in sys.modules:
        return
    try:
        import contextlib
        import ctypes
        import types

        mod = types.ModuleType("antenv.axon_hooks")
        _state = {"hook": None}

        def set_axon_ntff_profile_hook(hook):
            _state["hook"] = hook

        def get_axon_ntff_profile_hook():
            if _state["hook"] is None:
                try:
                    lib = ctypes.CDLL("/opt/axon/libaxon_pjrt.so")
                except OSError:
                    return None
                if not hasattr(lib, "axon_start_nrt_profile"):
                    return None
                lib.axon_start_nrt_profile.argtypes = [ctypes.POINTER(ctypes.c_int64), ctypes.c_size_t]
                lib.axon_start_nrt_profile.restype = ctypes.c_int64
                lib.axon_stop_nrt_profile.argtypes = [ctypes.c_char_p]
                lib.axon_stop_nrt_profile.restype = ctypes.c_int64

                @contextlib.contextmanager
                def _hook(output_dir, device_ids):
                    import jax

                    jax.devices()
                    if device_ids:
                        ids = (ctypes.c_int64 * len(device_ids))(*device_ids)
                        rc = lib.axon_start_nrt_profile(ids, len(device_ids))
                    else:
                        rc = lib.axon_start_nrt_profile(None, 0)
                    if rc != 0:
                        raise RuntimeError(f"axon_start_nrt_profile rc={rc}")
                    try:
                        yield
                    finally:
                        n = lib.axon_stop_nrt_profile(str(output_dir).encode())
                        print(f"profile: {n} file(s) written to {output_dir}", file=sys.stderr)

                _state["hook"] = _hook
            return _state["hook"]

        mod.set_axon_ntff_profile_hook = set_axon_ntff_profile_hook
        mod.get_axon_ntff_profile_hook = get_axon_ntff_profile_hook
        sys.modules["antenv.axon_hooks"] = mod
        try:
            import antenv

            antenv.axon_hooks = mod
        except ImportError:
            pass
    except Exception:
        pass


def run(inputs, trace=False, **kw):
    _ensure_ntff_hook_shim()
    from concourse import bass_utils as BU
    from concourse.bass_utils import run_bass_kernel_spmd

    if not getattr(BU.upload_artifacts, "_safe", False):
        _orig_upload = BU.upload_artifacts

        def _safe_upload(tmpdir):
            try:
                return _orig_upload(tmpdir)
            except Exception:
                return f"local:{tmpdir}"

        _safe_upload._safe = True
        BU.upload_artifacts = _safe_upload

    nc = _get_nc()
    in_maps = make_in_maps(**inputs)
    res = run_bass_kernel_spmd(nc, in_maps, list(range(H)), trace=trace, **kw)
    bo = np.asarray(inputs["bo"], np.float32)
    # V-bias contribution: softmax rows sum to 1, so attn = W@(x Wv^T) + 1*bv^T
    # and y gains the constant row Wo @ bv (exact). Added here with bo.
    yv = np.asarray(inputs["Wo"], np.float32) @ np.asarray(inputs["bv"], np.float32)
    out = np.zeros((B, D), np.float32)
    for c in range(H):
        out += np.asarray(res.results[c]["y"], dtype=np.float32)
    out += (bo + yv)[None, :]
    return out, res


def kernel(**inputs):
    out, _ = run(inputs, trace=False)
    return out


# revision 41
# speedup vs baseline: 1.4992x; 1.0788x over previous
"""Multi-head cross-batch attention (B=4096, d_model=512, H=8 heads) on 8 TRN2 cores.

Sharding: one head per NeuronCore (tensor-parallel over H). Each core computes
its head's Q/K/V projections from a replicated (pre-transposed) x, the full
[4096, 4096] score block for that head, softmax (transposed layout, denominator
via a ones-column in V), attn @ V, and its partial out-projection
Y_h = attn_h @ Wo[:, h*64:(h+1)*64].T. Host sums the 8 partials and adds bo.

Layout notes (per core):
  - xT [512, 4096] (c on partitions) is fed from host so every matmul can
    contract over the partition dim without any on-device transpose of x.
  - QT/KT are stored duplicated across partition halves ([128, 4096]) so score
    matmuls can be row-packed two-at-a-time into the 128x128 PE array (the
    contraction dim is only 64).
  - V is projected directly token-major (x chunk as the stationary operand).
    The V bias is dropped on device: softmax rows sum to 1, so bv contributes
    the constant row Wo @ bv to y, which the host adds together with bo
    (exact). Similarly the K bias only shifts each softmax row by a constant
    and could be dropped, but it is kept since the add rides a drain copy.
  - Scores are computed transposed (ST[j, m]) so softmax's sum over keys j can
    ride the attn@V matmul: V is augmented with a ones column, making the
    accumulated output row 64 equal to sum_j exp(s). No max-subtraction is
    needed: scores are O(1) here (verified), so exp cannot overflow.
  - Normalization commutes with the out-projection and is applied to the bf16
    numerator along its free dim: fp32 reciprocal of the r row, a single K=1
    broadcast matmul to replicate it across 64 partitions, one multiply.
  - Matmul inputs are bf16 (1 PE pass vs 2 for fp32, fast weight load);
    accumulation is fp32 in PSUM, exp inputs and the softmax denominator stay
    fp32. y partials return in bf16 (host accumulates in fp32).
  - A short warm-up matmul stream runs while x streams in, so the PE's HAM
    clock gate reaches 8/8 before the first projection matmul.
  - Emission is software-pipelined: attnV trails its scores/exp by one group so
    the PE's in-order queue never blocks on an in-flight exp; each chunk's
    output phase is emitted mid-way through the next chunk; chunk 0's score
    groups are interleaved into the projection loop as x arrives n-major;
    chunk n's Q projection is deferred into chunk n-1's group stream.
"""

import sys
from collections import deque

if "/opt/trn_rl_repo" not in sys.path:
    sys.path.insert(0, "/opt/trn_rl_repo")

import ml_dtypes
import numpy as np

import concourse.bass as bass
import concourse.tile as tile
from concourse import bacc, mybir

B = 4096
D = 512
H = 8
DK = 64
MC = 512  # query-chunk (m) width
N_MC = B // MC  # 8
JB = B // 128  # 32 j-blocks of 128 keys
F32 = mybir.dt.float32
BF16 = mybir.dt.bfloat16
MM_DT = BF16
NP_MM_DT = ml_dtypes.bfloat16 if MM_DT == BF16 else np.float32

# packed weight layout: per c-chunk of 128 channels, [wq_dup(128) | wk_dup(128) | wv(64)]
WCOLS = 128 + 128 + DK  # 320

# j-blocks per score/exp group: 3 blocks = 1536 floats = 3 PSUM banks.
# PSUM budget: 2x3 (score staging) + 1 (attnV accum) + 1 (vproj/outproj) = 8.
JGROUPS = [(0, 3), (3, 3), (6, 3), (9, 3), (12, 3), (15, 3), (18, 3), (21, 3), (24, 3), (27, 3), (30, 2)]

_NC_CACHE = None


def build_nc():
    nc = bacc.Bacc()

    xt = nc.dram_tensor("xt", [D, B], MM_DT, kind="ExternalInput")
    w3 = nc.dram_tensor("w3", [D, WCOLS], MM_DT, kind="ExternalInput")  # [c, wq|wk|wv]
    b3 = nc.dram_tensor("b3", [128, 2], F32, kind="ExternalInput")  # bq_dup | bk_dup
    wot = nc.dram_tensor("wot", [DK, D], MM_DT, kind="ExternalInput")
    y = nc.dram_tensor("y", [B, D], MM_DT, kind="ExternalOutput")

    with tile.TileContext(nc) as tc:
        with (
            tc.tile_pool(name="const", bufs=1) as const,
            tc.tile_pool(name="epool", bufs=20) as epool,
            tc.tile_pool(name="otpool", bufs=3) as otpool,
            tc.tile_pool(name="ypool", bufs=3) as ypool,
            tc.tile_pool(name="score_ps", bufs=2, space="PSUM") as score_ps,
            tc.tile_pool(name="attnv_ps", bufs=1, space="PSUM") as attnv_ps,
            tc.tile_pool(name="out_ps", bufs=1, space="PSUM") as out_ps,
        ):
            # ---- persistent SBUF ----
            x_sb = const.tile([128, 4 * B], MM_DT)  # 4 c-chunks side by side
            w3_sb = const.tile([128, 4 * WCOLS], MM_DT)  # 4 c-chunks of [128,320]
            b3_sb = const.tile([128, 2], F32)
            wot_sb = const.tile([DK, D], MM_DT)
            warm_sb = const.tile([128, MC], MM_DT)
            ones65 = const.tile([65, DK], MM_DT)  # row 64 = ones (K=1 broadcast matmul lhsT)
            qt_sb = const.tile([128, B], MM_DT)  # QT dup'd across partition halves
            kt_sb = const.tile([128, B], MM_DT)
            vp_sb = const.tile([128, JB * (DK + 1)], MM_DT)  # [V | 1] per j-block

            # ---- input DMAs ----
            # Co-queued DMAs on one ring complete nearly together, so the
            # weights get the scalar HWDGE ring to themselves (land first),
            # x chunk 0 gets the sync ring, and the rest of x spreads over all
            # three rings roughly in consumption order.
            x_sb3 = x_sb[:].rearrange("p (c n) -> p c n", c=4)
            xt3 = xt[:].rearrange("(c p) n -> p c n", p=128)
            nc.scalar.dma_start(
                out=w3_sb[:].rearrange("p (c n) -> p c n", c=4),
                in_=w3[:].rearrange("(c p) n -> p c n", p=128),
            )
            nc.scalar.dma_start(out=b3_sb[:], in_=b3[:])
            nc.sync.dma_start(out=x_sb3[:, :, 0:MC], in_=xt3[:, :, 0:MC])
            nc.sync.dma_start(out=wot_sb[:], in_=wot[:])
            x_dma_eng = {1: nc.gpsimd, 2: nc.sync, 3: nc.scalar, 4: nc.gpsimd, 5: nc.sync, 6: nc.scalar, 7: nc.gpsimd}
            for n in range(1, N_MC):
                x_dma_eng[n].dma_start(
                    out=x_sb3[:, :, n * MC : (n + 1) * MC],
                    in_=xt3[:, :, n * MC : (n + 1) * MC],
                )
            nc.vector.memset(warm_sb[:], 0.125)
            nc.vector.memset(ones65[DK : DK + 1, :], 1.0)
            # ones columns of the augmented-V layout
            nc.vector.memset(
                vp_sb[:].rearrange("p (t e) -> p t e", e=DK + 1)[:, :, DK : DK + 1], 1.0
            )
            # preload the exp activation table set before the first real exp
            # (junk output target so nothing downstream depends on it)
            junk_sb = const.tile([65, 4], F32)
            nc.scalar.activation(
                junk_sb[DK : DK + 1, :], ones65[DK : DK + 1, 0:4], mybir.ActivationFunctionType.Exp, scale=0.125
            )

            # ---- PE warm-up: long-stream matmuls (high busy duty) while x
            # streams in, so the HAM clock gate is at 8/8 when the first
            # projection matmul runs.
            for _ in range(10):
                wp = out_ps.tile([DK, MC], F32, tag="out")
                nc.tensor.matmul(wp[:], warm_sb[:, 0:DK], warm_sb[:], start=True, stop=True)

            # ---- emission helpers ----
            def emit_proj(n, off, pool=None, tag="score"):
                # one staging-ring unit holds one projection chunk [128, 512].
                # off 0 -> Q (w3 cols 0:128, bias col 0), off 1 -> K (cols 128:256, bias col 1)
                pp = (pool or score_ps).tile([128, MC], F32, tag=tag)
                for c in range(4):
                    nc.tensor.matmul(
                        pp[:],
                        w3_sb[:, c * WCOLS + off * 128 : c * WCOLS + off * 128 + 128],
                        x_sb[:, c * B + n * MC : c * B + (n + 1) * MC],
                        start=(c == 0),
                        stop=(c == 3),
                    )
                dst = qt_sb if off == 0 else kt_sb
                nc.vector.tensor_scalar(
                    out=dst[:, n * MC : (n + 1) * MC], in0=pp[:],
                    scalar1=b3_sb[:, off : off + 1], scalar2=None, op0=mybir.AluOpType.add,
                )

            def emit_v4(n):
                # V for j-blocks 4n..4n+3 token-major (x chunk stationary), no
                # bias (folded into a host-side constant), one strided drain.
                t0 = 4 * n
                vps = out_ps.tile([128, 4 * DK], F32, tag="out")
                for k in range(4):
                    dst = vps[:, k * DK : (k + 1) * DK]
                    for c in range(4):
                        nc.tensor.matmul(
                            dst,
                            x_sb[:, c * B + (t0 + k) * 128 : c * B + (t0 + k + 1) * 128],
                            w3_sb[:, c * WCOLS + 256 : c * WCOLS + 256 + DK],
                            start=(c == 0),
                            stop=(c == 3),
                        )
                nc.vector.tensor_copy(
                    vp_sb[:].rearrange("p (t e) -> p t e", e=DK + 1)[:, t0 : t0 + 4, 0:DK],
                    vps[:].rearrange("p (t e) -> p t e", e=DK),
                )

            def emit_sc(mc, g0, gn):
                m0 = mc * MC
                sp = score_ps.tile([128, gn * MC], F32, tag="score")
                et = epool.tile([128, gn * MC], MM_DT, tag="E")
                for k in range(gn):
                    jb = g0 + k
                    h0 = 64 * (jb % 2)
                    nc.tensor.matmul(
                        sp[:, k * MC : (k + 1) * MC],
                        kt_sb[h0 : h0 + 64, jb * 128 : (jb + 1) * 128],
                        qt_sb[h0 : h0 + 64, m0 : m0 + MC],
                        start=True,
                        stop=True,
                    )
                nc.scalar.activation(et[:], sp[:], mybir.ActivationFunctionType.Exp, scale=0.125)
                return et

            def emit_av(mc, g0, gn, et, av):
                for k in range(gn):
                    jb = g0 + k
                    nc.tensor.matmul(
                        av[:],
                        vp_sb[:, jb * (DK + 1) : (jb + 1) * (DK + 1)],
                        et[:, k * MC : (k + 1) * MC],
                        start=(jb == 0),
                        stop=(jb == JB - 1),
                    )

            def emit_otcopy(av):
                # Phase 1 of the output path: free av with FAST copies only
                # (single-lane reciprocal costs ~3.4us on DVE — measured — so
                # it must not hold av or the PE queue). The reciprocal runs
                # asynchronously on the copied r row; phase 2 is deferred
                # several drain slots to let it finish.
                otb = otpool.tile([DK, MC], MM_DT, tag="ot")
                nc.vector.tensor_copy(otb[:], av[0:DK, :])
                rr = otpool.tile([DK + 1, MC], F32, tag="rrow")
                nc.vector.tensor_copy(rr[DK : DK + 1, :], av[DK : DK + 1, :])
                ri = otpool.tile([DK + 1, MC], MM_DT, tag="rinv")
                with nc.allow_low_precision(reason="softmax denominators are O(1); bf16 1/r costs ~0.2% rel"):
                    nc.vector.reciprocal(ri[DK : DK + 1, :], rr[DK : DK + 1, :])
                return ri, otb

            def emit_norm(rr, otb):
                # Phase 2 (one group later, reciprocal long done): K=1 bf16
                # matmul broadcasts 1/r across 64 partitions; normalize the
                # numerator along its free (query) dim.
                rb = out_ps.tile([DK, MC], F32, tag="out")
                nc.tensor.matmul(rb[:], ones65[DK : DK + 1, :], rr[DK : DK + 1, :], start=True, stop=True)
                otn = otpool.tile([DK, MC], MM_DT, tag="otn")
                nc.vector.scalar_tensor_tensor(
                    out=otn[:], in0=otb[:], scalar=1.0, in1=rb[:],
                    op0=mybir.AluOpType.mult, op1=mybir.AluOpType.mult,
                )
                return otn

            def emit_output(mc, otn):
                m0 = mc * MC
                ysb = ypool.tile([128, 4 * MC], MM_DT, tag="y")
                for q in range(4):
                    yp = out_ps.tile([128, MC], F32, tag="out")
                    nc.tensor.matmul(yp[:], otn[:, q * 128 : (q + 1) * 128], wot_sb[:], start=True, stop=True)
                    nc.vector.tensor_copy(ysb[:, q * MC : (q + 1) * MC], yp[:])
                    if q == 1:
                        nc.gpsimd.dma_start(
                            out=y[m0 : m0 + 256, :].rearrange("(q p) d -> p q d", p=128),
                            in_=ysb[:, 0 : 2 * MC].rearrange("p (q d) -> p q d", q=2),
                        )
                nc.sync.dma_start(
                    out=y[m0 + 256 : m0 + MC, :].rearrange("(q p) d -> p q d", p=128),
                    in_=ysb[:, 2 * MC : 4 * MC].rearrange("p (q d) -> p q d", q=2),
                )

            # ---- software-pipelined main emission ----
            # attnV for a group is emitted one group behind its scores/exp, so
            # the PE's in-order queue never blocks on an in-flight exp. Each
            # chunk's output phase is emitted mid-way through the next chunk.
            # Chunk n's Q projection is emitted mid-way through chunk n-1.
            state = {"q": deque(), "av": None, "norm_pending": None, "out_pending": None, "vn": 0}

            def drain_one():
                mc, g0, gn, et = state["q"].popleft()
                # just-in-time V projection: blocks g0..g0+gn-1 (+1 chunk ahead)
                while state["vn"] < N_MC and state["vn"] <= (g0 + gn - 1) // 4 + 1:
                    emit_v4(state["vn"])
                    state["vn"] += 1
                if state["av"] is None:
                    state["av"] = attnv_ps.tile([DK + 1, MC], F32, tag="attnv", name="av")
                emit_av(mc, g0, gn, et, state["av"])
                if state["norm_pending"] is not None:
                    pmc, ri, otb, cnt = state["norm_pending"]
                    if cnt <= 0:
                        state["norm_pending"] = None
                        state["out_pending"] = (pmc, emit_norm(ri, otb))
                    else:
                        state["norm_pending"] = (pmc, ri, otb, cnt - 1)
                if g0 + gn == JB:  # chunk complete: free av (fast DVE copies)
                    state["norm_pending"] = (mc, *emit_otcopy(state["av"]), 3)
                    state["av"] = None
                elif g0 >= 12 and state["out_pending"] is not None and state["out_pending"][0] == mc - 1:
                    emit_output(*state["out_pending"])
                    state["out_pending"] = None

            def push(mc, g0, gn):
                # attnV trails scores/exp by TRAIL(mc) groups: deep for chunk 0
                # (spills chunk-0 attnV into later chunks' PE slack while exp
                # tracks the incoming x stream), shallow in steady state.
                et = emit_sc(mc, g0, gn)
                state["q"].append((mc, g0, gn, et))
                trail = {0: 6, 1: 4, 2: 2}.get(mc, 1)
                while len(state["q"]) > trail:
                    drain_one()

            # projections interleaved with chunk 0 (x arrives n-major).
            # K and V are needed globally and are emitted as x arrives; Q is
            # needed per-chunk: Q0/Q1 up front, Qn during chunk n-1.
            gi = 0
            for n in range(N_MC):
                emit_proj(n, 1)  # K
                if n <= 1:
                    emit_proj(n, 0)  # Q0, Q1
                while gi < len(JGROUPS) and JGROUPS[gi][0] + JGROUPS[gi][1] <= 4 * n + 4:
                    push(0, *JGROUPS[gi])
                    gi += 1
            while gi < len(JGROUPS):
                push(0, *JGROUPS[gi])
                gi += 1
            for mc in range(1, N_MC):
                for ig, (g0, gn) in enumerate(JGROUPS):
                    push(mc, g0, gn)
                    if ig == 5 and mc + 1 < N_MC:
                        # Q for the next chunk; staged in out_ps so the score
                        # ring's double-buffer parity is undisturbed.
                        emit_proj(mc + 1, 0, pool=out_ps, tag="out")
            while state["q"]:
                drain_one()
            pmc, ri, otb, _ = state["norm_pending"]
            emit_output(pmc, emit_norm(ri, otb))
    nc.finalize()
    return nc


def _get_nc():
    global _NC_CACHE
    if _NC_CACHE is None:
        _NC_CACHE = build_nc()
    return _NC_CACHE


def make_in_maps(x, Wq, bq, Wk, bk, Wv, bv, Wo, bo):
    xT = np.ascontiguousarray(np.asarray(x, dtype=np.float32).T).astype(NP_MM_DT)
    maps = []
    for h in range(H):
        s = slice(h * DK, (h + 1) * DK)
        wqT = np.asarray(Wq, np.float32)[s, :].T  # [512, 64]
        wkT = np.asarray(Wk, np.float32)[s, :].T
        wvT = np.asarray(Wv, np.float32)[s, :].T
        w3 = np.concatenate([wqT, wqT, wkT, wkT, wvT], axis=1)  # [512, 320]
        b3 = np.zeros((128, 2), np.float32)
        b3[:, 0] = np.tile(np.asarray(bq, np.float32)[s], 2)
        b3[:, 1] = np.tile(np.asarray(bk, np.float32)[s], 2)
        maps.append(
            {
                "xt": xT,
                "w3": np.ascontiguousarray(w3).astype(NP_MM_DT),
                "b3": b3,
                "wot": np.ascontiguousarray(np.asarray(Wo, np.float32)[:, s].T).astype(NP_MM_DT),
            }
        )
    return maps


def _ensure_ntff_hook_shim():
    # The image's antenv package lacks axon_hooks; bass_utils imports it when
    # tracing is requested (including via the BASS_TRACE env var). Register a
    # ctypes-backed shim so that path works regardless of environment.
    if "antenv.axon_hooks" in sys.modules:
        return
    try:
        import contextlib
        import ctypes
        import types

        mod = types.ModuleType("antenv.axon_hooks")
        _state = {"hook": None}

        def set_axon_ntff_profile_hook(hook):
            _state["hook"] = hook

        def get_axon_ntff_profile_hook():
            if _state["hook"] is None:
                try:
                    lib = ctypes.CDLL("/opt/axon/libaxon_pjrt.so")
                except OSError:
                    return None
                if not hasattr(lib, "axon_start_nrt_profile"):
                    return None
                lib.axon_start_nrt_profile.argtypes = [ctypes.POINTER(ctypes.c_int64), ctypes.c_size_t]
                lib.axon_start_nrt_profile.restype = ctypes.c_int64
                lib.axon_stop_nrt_profile.argtypes = [ctypes.c_char_p]
                lib.axon_stop_nrt_profile.restype = ctypes.c_int64

                @contextlib.contextmanager
                def _hook(output_dir, device_ids):
                    import jax

                    jax.devices()
                    if device_ids:
                        ids = (ctypes.c_int64 * len(device_ids))(*device_ids)
                        rc = lib.axon_start_nrt_profile(ids, len(device_ids))
                    else:
                        rc = lib.axon_start_nrt_profile(None, 0)
                    if rc != 0:
                        raise RuntimeError(f"axon_start_nrt_profile rc={rc}")
                    try:
                        yield
                    finally:
                        n = lib.axon_stop_nrt_profile(str(output_dir).encode())
                        print(f"profile: {n} file(s) written to {output_dir}", file=sys.stderr)

                _state["hook"] = _hook
            return _state["hook"]

        mod.set_axon_ntff_profile_hook = set_axon_ntff_profile_hook
        mod.get_axon_ntff_profile_hook = get_axon_ntff_profile_hook
        sys.modules["antenv.axon_hooks"] = mod
        try:
            import antenv

            antenv.axon_hooks = mod
        except ImportError:
            pass
    except Exception:
        pass


def run(inputs, trace=False, **kw):
    _ensure_ntff_hook_shim()
    from concourse import bass_utils as BU
    from concourse.bass_utils import run_bass_kernel_spmd

    if not getattr(BU.upload_artifacts, "_safe", False):
        _orig_upload = BU.upload_artifacts

        def _safe_upload(tmpdir):
            try:
                return _orig_upload(tmpdir)
            except Exception:
                return f"local:{tmpdir}"

        _safe_upload._safe = True
        BU.upload_artifacts = _safe_upload

    nc = _get_nc()
    in_maps = make_in_maps(**inputs)
    res = run_bass_kernel_spmd(nc, in_maps, list(range(H)), trace=trace, **kw)
    bo = np.asarray(inputs["bo"], np.float32)
    # V-bias contribution: softmax rows sum to 1, so attn = W@(x Wv^T) + 1*bv^T
    # and y gains the constant row Wo @ bv (exact). Added here with bo.
    yv = np.asarray(inputs["Wo"], np.float32) @ np.asarray(inputs["bv"], np.float32)
    out = np.zeros((B, D), np.float32)
    for c in range(H):
        out += np.asarray(res.results[c]["y"], dtype=np.float32)
    out += (bo + yv)[None, :]
    return out, res


def kernel(**inputs):
    out, _ = run(inputs, trace=False)
    return out


# revision 49
# speedup vs baseline: 1.5503x; 1.0341x over previous
"""Multi-head cross-batch attention (B=4096, d_model=512, H=8 heads) on 8 TRN2 cores.

Sharding: one head per NeuronCore (tensor-parallel over H). Each core computes
its head's Q/K/V projections from a replicated (pre-transposed) x, the full
[4096, 4096] score block for that head, softmax (transposed layout, denominator
via a ones-column in V), attn @ V, and its partial out-projection
Y_h = attn_h @ Wo[:, h*64:(h+1)*64].T. Host sums the 8 partials and adds bo.

Layout notes (per core):
  - xT [512, 4096] (c on partitions) is fed from host so every matmul can
    contract over the partition dim without any on-device transpose of x.
  - QT/KT are stored duplicated across partition halves ([128, 4096]) so score
    matmuls can be row-packed two-at-a-time into the 128x128 PE array (the
    contraction dim is only 64).
  - V is projected directly token-major (x chunk as the stationary operand).
    The V bias is dropped on device: softmax rows sum to 1, so bv contributes
    the constant row Wo @ bv to y, which the host adds together with bo
    (exact). Similarly the K bias only shifts each softmax row by a constant
    and could be dropped, but it is kept since the add rides a drain copy.
  - Scores are computed transposed (ST[j, m]) so softmax's sum over keys j can
    ride the attn@V matmul: V is augmented with a ones column, making the
    accumulated output row 64 equal to sum_j exp(s). No max-subtraction is
    needed: scores are O(1) here (verified), so exp cannot overflow.
  - Normalization commutes with the out-projection and is applied to the bf16
    numerator along its free dim: fp32 reciprocal of the r row, a single K=1
    broadcast matmul to replicate it across 64 partitions, one multiply.
  - Matmul inputs are bf16 (1 PE pass vs 2 for fp32, fast weight load);
    accumulation is fp32 in PSUM, exp inputs and the softmax denominator stay
    fp32. y partials return in bf16 (host accumulates in fp32).
  - A short warm-up matmul stream runs while x streams in, so the PE's HAM
    clock gate reaches 8/8 before the first projection matmul.
  - Emission is software-pipelined: attnV trails its scores/exp by one group so
    the PE's in-order queue never blocks on an in-flight exp; each chunk's
    output phase is emitted mid-way through the next chunk; chunk 0's score
    groups are interleaved into the projection loop as x arrives n-major;
    chunk n's Q projection is deferred into chunk n-1's group stream.
"""

import sys
from collections import deque

if "/opt/trn_rl_repo" not in sys.path:
    sys.path.insert(0, "/opt/trn_rl_repo")

import ml_dtypes
import numpy as np

import concourse.bass as bass
import concourse.tile as tile
from concourse import bacc, mybir

B = 4096
D = 512
H = 8
DK = 64
MC = 512  # query-chunk (m) width
N_MC = B // MC  # 8
JB = B // 128  # 32 j-blocks of 128 keys
F32 = mybir.dt.float32
BF16 = mybir.dt.bfloat16
MM_DT = BF16
NP_MM_DT = ml_dtypes.bfloat16 if MM_DT == BF16 else np.float32

# packed weight layout: per c-chunk of 128 channels, [wq_dup(128) | wk_dup(128) | wv(64)]
WCOLS = 128 + 128 + DK  # 320

# j-blocks per score/exp group: 3 blocks = 1536 floats = 3 PSUM banks.
# PSUM budget: 2x3 (score staging) + 1 (attnV accum) + 1 (vproj/outproj) = 8.
JGROUPS = [(0, 3), (3, 3), (6, 3), (9, 3), (12, 3), (15, 3), (18, 3), (21, 3), (24, 3), (27, 3), (30, 2)]

_NC_CACHE = None


def build_nc():
    nc = bacc.Bacc()

    xt = nc.dram_tensor("xt", [D, B], MM_DT, kind="ExternalInput")
    w3 = nc.dram_tensor("w3", [D, WCOLS], MM_DT, kind="ExternalInput")  # [c, wq|wk|wv]
    b3 = nc.dram_tensor("b3", [128, 2], F32, kind="ExternalInput")  # bq_dup | bk_dup
    wot = nc.dram_tensor("wot", [DK, D], MM_DT, kind="ExternalInput")
    y = nc.dram_tensor("y", [B, D], MM_DT, kind="ExternalOutput")

    with tile.TileContext(nc) as tc:
        with (
            tc.tile_pool(name="const", bufs=1) as const,
            tc.tile_pool(name="epool", bufs=20) as epool,
            tc.tile_pool(name="otpool", bufs=3) as otpool,
            tc.tile_pool(name="ypool", bufs=3) as ypool,
            tc.tile_pool(name="score_ps", bufs=2, space="PSUM") as score_ps,
            tc.tile_pool(name="attnv_ps", bufs=1, space="PSUM") as attnv_ps,
            tc.tile_pool(name="out_ps", bufs=1, space="PSUM") as out_ps,
        ):
            # ---- persistent SBUF ----
            x_sb = const.tile([128, 4 * B], MM_DT)  # 4 c-chunks side by side
            w3_sb = const.tile([128, 4 * WCOLS], MM_DT)  # 4 c-chunks of [128,320]
            b3_sb = const.tile([128, 2], F32)
            wot_sb = const.tile([DK, D], MM_DT)
            warm_sb = const.tile([128, MC], MM_DT)
            ones65 = const.tile([65, DK], MM_DT)  # row 64 = ones (K=1 broadcast matmul lhsT)
            qt_sb = const.tile([128, B], MM_DT)  # QT dup'd across partition halves
            kt_sb = const.tile([128, B], MM_DT)
            vp_sb = const.tile([128, JB * (DK + 1)], MM_DT)  # [V | 1] per j-block

            # ---- input DMAs ----
            # Co-queued DMAs on one ring complete nearly together, so the
            # weights get the scalar HWDGE ring to themselves (land first),
            # x chunk 0 gets the sync ring, and the rest of x spreads over all
            # three rings roughly in consumption order.
            x_sb3 = x_sb[:].rearrange("p (c n) -> p c n", c=4)
            xt3 = xt[:].rearrange("(c p) n -> p c n", p=128)
            nc.scalar.dma_start(
                out=w3_sb[:].rearrange("p (c n) -> p c n", c=4),
                in_=w3[:].rearrange("(c p) n -> p c n", p=128),
            )
            nc.scalar.dma_start(out=b3_sb[:], in_=b3[:])
            nc.sync.dma_start(out=x_sb3[:, :, 0:MC], in_=xt3[:, :, 0:MC])
            nc.sync.dma_start(out=wot_sb[:], in_=wot[:])
            x_dma_eng = {1: nc.gpsimd, 2: nc.sync, 3: nc.scalar, 4: nc.gpsimd, 5: nc.sync, 6: nc.scalar, 7: nc.gpsimd}
            for n in range(1, N_MC):
                x_dma_eng[n].dma_start(
                    out=x_sb3[:, :, n * MC : (n + 1) * MC],
                    in_=xt3[:, :, n * MC : (n + 1) * MC],
                )
            nc.vector.memset(warm_sb[:], 0.125)
            nc.vector.memset(ones65[DK : DK + 1, :], 1.0)
            # ones columns of the augmented-V layout
            nc.vector.memset(
                vp_sb[:].rearrange("p (t e) -> p t e", e=DK + 1)[:, :, DK : DK + 1], 1.0
            )
            # preload the exp activation table set before the first real exp
            # (junk output target so nothing downstream depends on it)
            junk_sb = const.tile([65, 4], F32)
            nc.scalar.activation(
                junk_sb[DK : DK + 1, :], ones65[DK : DK + 1, 0:4], mybir.ActivationFunctionType.Exp, scale=0.125
            )

            # ---- PE warm-up: long-stream matmuls (high busy duty) while x
            # streams in, so the HAM clock gate is at 8/8 when the first
            # projection matmul runs.
            for _ in range(10):
                wp = out_ps.tile([DK, MC], F32, tag="out")
                nc.tensor.matmul(wp[:], warm_sb[:, 0:DK], warm_sb[:], start=True, stop=True)

            # ---- emission helpers ----
            def emit_proj(n, off, pool=None, tag="score"):
                # one staging-ring unit holds one projection chunk [128, 512].
                # off 0 -> Q (w3 cols 0:128, bias col 0), off 1 -> K (cols 128:256, bias col 1)
                pp = (pool or score_ps).tile([128, MC], F32, tag=tag)
                for c in range(4):
                    nc.tensor.matmul(
                        pp[:],
                        w3_sb[:, c * WCOLS + off * 128 : c * WCOLS + off * 128 + 128],
                        x_sb[:, c * B + n * MC : c * B + (n + 1) * MC],
                        start=(c == 0),
                        stop=(c == 3),
                    )
                dst = qt_sb if off == 0 else kt_sb
                nc.vector.tensor_scalar(
                    out=dst[:, n * MC : (n + 1) * MC], in0=pp[:],
                    scalar1=b3_sb[:, off : off + 1], scalar2=None, op0=mybir.AluOpType.add,
                )

            def emit_v4(n):
                # V for j-blocks 4n..4n+3 token-major (x chunk stationary), no
                # bias (folded into a host-side constant), one strided drain.
                t0 = 4 * n
                vps = out_ps.tile([128, 4 * DK], F32, tag="out")
                for k in range(4):
                    dst = vps[:, k * DK : (k + 1) * DK]
                    for c in range(4):
                        nc.tensor.matmul(
                            dst,
                            x_sb[:, c * B + (t0 + k) * 128 : c * B + (t0 + k + 1) * 128],
                            w3_sb[:, c * WCOLS + 256 : c * WCOLS + 256 + DK],
                            start=(c == 0),
                            stop=(c == 3),
                        )
                nc.vector.tensor_copy(
                    vp_sb[:].rearrange("p (t e) -> p t e", e=DK + 1)[:, t0 : t0 + 4, 0:DK],
                    vps[:].rearrange("p (t e) -> p t e", e=DK),
                )

            def emit_sc(mc, g0, gn):
                m0 = mc * MC
                sp = score_ps.tile([128, gn * MC], F32, tag="score")
                et = epool.tile([128, gn * MC], MM_DT, tag="E")
                for k in range(gn):
                    jb = g0 + k
                    h0 = 64 * (jb % 2)
                    nc.tensor.matmul(
                        sp[:, k * MC : (k + 1) * MC],
                        kt_sb[h0 : h0 + 64, jb * 128 : (jb + 1) * 128],
                        qt_sb[h0 : h0 + 64, m0 : m0 + MC],
                        start=True,
                        stop=True,
                    )
                nc.scalar.activation(et[:], sp[:], mybir.ActivationFunctionType.Exp, scale=0.125)
                return et

            def emit_av(mc, g0, gn, et, av):
                for k in range(gn):
                    jb = g0 + k
                    nc.tensor.matmul(
                        av[:],
                        vp_sb[:, jb * (DK + 1) : (jb + 1) * (DK + 1)],
                        et[:, k * MC : (k + 1) * MC],
                        start=(jb == 0),
                        stop=(jb == JB - 1),
                    )

            def emit_otcopy(av):
                # Phase 1 of the output path: free av with FAST copies only
                # (single-lane reciprocal costs ~3.4us on DVE — measured — so
                # it must not hold av or the PE queue). The reciprocal runs
                # asynchronously on the copied r row; phase 2 is deferred
                # several drain slots to let it finish.
                otb = otpool.tile([DK, MC], MM_DT, tag="ot")
                nc.vector.tensor_copy(otb[:], av[0:DK, :])
                rr = otpool.tile([DK + 1, MC], MM_DT, tag="rrow")
                with nc.allow_low_precision(reason="softmax denominators are O(1); bf16 r costs ~0.2% rel"):
                    nc.vector.tensor_copy(rr[DK : DK + 1, :], av[DK : DK + 1, :])
                return rr, otb

            def emit_rinv(rr):
                ri = otpool.tile([DK + 1, MC], MM_DT, tag="rinv")
                with nc.allow_low_precision(reason="softmax denominators are O(1); bf16 1/r costs ~0.2% rel"):
                    nc.vector.reciprocal(ri[DK : DK + 1, :], rr[DK : DK + 1, :])
                return ri

            def emit_norm(rr, otb):
                # Phase 2 (one group later, reciprocal long done): K=1 bf16
                # matmul broadcasts 1/r across 64 partitions; normalize the
                # numerator along its free (query) dim.
                rb = out_ps.tile([DK, MC], F32, tag="out")
                nc.tensor.matmul(rb[:], ones65[DK : DK + 1, :], rr[DK : DK + 1, :], start=True, stop=True)
                otn = otpool.tile([DK, MC], MM_DT, tag="otn")
                nc.vector.scalar_tensor_tensor(
                    out=otn[:], in0=otb[:], scalar=1.0, in1=rb[:],
                    op0=mybir.AluOpType.mult, op1=mybir.AluOpType.mult,
                )
                return otn

            def emit_output(mc, otn):
                m0 = mc * MC
                ysb = ypool.tile([128, 4 * MC], MM_DT, tag="y")
                for q in range(4):
                    yp = out_ps.tile([128, MC], F32, tag="out")
                    nc.tensor.matmul(yp[:], otn[:, q * 128 : (q + 1) * 128], wot_sb[:], start=True, stop=True)
                    nc.vector.tensor_copy(ysb[:, q * MC : (q + 1) * MC], yp[:])
                    if q == 1:
                        nc.gpsimd.dma_start(
                            out=y[m0 : m0 + 256, :].rearrange("(q p) d -> p q d", p=128),
                            in_=ysb[:, 0 : 2 * MC].rearrange("p (q d) -> p q d", q=2),
                        )
                nc.sync.dma_start(
                    out=y[m0 + 256 : m0 + MC, :].rearrange("(q p) d -> p q d", p=128),
                    in_=ysb[:, 2 * MC : 4 * MC].rearrange("p (q d) -> p q d", q=2),
                )

            def emit_output_rt(mc, otb, rr):
                # Tail-path output (last chunk): per-q-block K=1 transpose of
                # the r row + 128-lane [128,1] reciprocals, scale after the
                # out-projection — avoids waiting on the slow row reciprocal.
                m0 = mc * MC
                ysb = ypool.tile([128, 4 * MC], MM_DT, tag="y")
                for q in range(4):
                    rt = out_ps.tile([128, MC], F32, tag="out")
                    nc.tensor.matmul(
                        rt[:, 0:1], rr[DK : DK + 1, q * 128 : (q + 1) * 128],
                        ones65[DK : DK + 1, 0:1], start=True, stop=True,
                    )
                    rv = otpool.tile([128, 1], F32, tag="rv")
                    nc.vector.reciprocal(rv[:], rt[:, 0:1])
                    yp = out_ps.tile([128, MC], F32, tag="out")
                    nc.tensor.matmul(yp[:], otb[:, q * 128 : (q + 1) * 128], wot_sb[:], start=True, stop=True)
                    nc.vector.tensor_scalar(
                        out=ysb[:, q * MC : (q + 1) * MC], in0=yp[:],
                        scalar1=rv[:], scalar2=None, op0=mybir.AluOpType.mult,
                    )
                    if q == 1:
                        nc.gpsimd.dma_start(
                            out=y[m0 : m0 + 256, :].rearrange("(q p) d -> p q d", p=128),
                            in_=ysb[:, 0 : 2 * MC].rearrange("p (q d) -> p q d", q=2),
                        )
                nc.sync.dma_start(
                    out=y[m0 + 256 : m0 + MC, :].rearrange("(q p) d -> p q d", p=128),
                    in_=ysb[:, 2 * MC : 4 * MC].rearrange("p (q d) -> p q d", q=2),
                )

            # ---- software-pipelined main emission ----
            # attnV for a group is emitted one group behind its scores/exp, so
            # the PE's in-order queue never blocks on an in-flight exp. Each
            # chunk's output phase is emitted mid-way through the next chunk.
            # Chunk n's Q projection is emitted mid-way through chunk n-1.
            state = {"q": deque(), "av": None, "norm_pending": None, "out_pending": None, "tail_pending": None, "vn": 0}

            def drain_one():
                mc, g0, gn, et = state["q"].popleft()
                # just-in-time V projection: blocks g0..g0+gn-1 (+1 chunk ahead)
                while state["vn"] < N_MC and state["vn"] <= (g0 + gn - 1) // 4 + 1:
                    emit_v4(state["vn"])
                    state["vn"] += 1
                if state["av"] is None:
                    state["av"] = attnv_ps.tile([DK + 1, MC], F32, tag="attnv", name="av")
                emit_av(mc, g0, gn, et, state["av"])
                if state["norm_pending"] is not None:
                    pmc, ri, otb, cnt = state["norm_pending"]
                    if cnt <= 0:
                        state["norm_pending"] = None
                        state["out_pending"] = (pmc, emit_norm(ri, otb))
                    else:
                        state["norm_pending"] = (pmc, ri, otb, cnt - 1)
                if g0 + gn == JB:  # chunk complete: free av (fast DVE copies)
                    rr, otb = emit_otcopy(state["av"])
                    state["av"] = None
                    if mc == N_MC - 1:
                        state["tail_pending"] = (mc, otb, rr)
                    else:
                        state["norm_pending"] = (mc, emit_rinv(rr), otb, 6)
                elif g0 >= 12 and state["out_pending"] is not None and state["out_pending"][0] == mc - 1:
                    emit_output(*state["out_pending"])
                    state["out_pending"] = None

            def push(mc, g0, gn):
                # attnV trails scores/exp by TRAIL(mc) groups: deep for chunk 0
                # (spills chunk-0 attnV into later chunks' PE slack while exp
                # tracks the incoming x stream), shallow in steady state.
                et = emit_sc(mc, g0, gn)
                state["q"].append((mc, g0, gn, et))
                trail = {0: 6, 1: 4, 2: 2}.get(mc, 1)
                while len(state["q"]) > trail:
                    drain_one()

            # projections interleaved with chunk 0 (x arrives n-major).
            # K and V are needed globally and are emitted as x arrives; Q is
            # needed per-chunk: Q0/Q1 up front, Qn during chunk n-1.
            gi = 0
            for n in range(N_MC):
                emit_proj(n, 1)  # K
                if n <= 1:
                    emit_proj(n, 0)  # Q0, Q1
                while gi < len(JGROUPS) and JGROUPS[gi][0] + JGROUPS[gi][1] <= 4 * n + 4:
                    push(0, *JGROUPS[gi])
                    gi += 1
            while gi < len(JGROUPS):
                push(0, *JGROUPS[gi])
                gi += 1
            for mc in range(1, N_MC):
                for ig, (g0, gn) in enumerate(JGROUPS):
                    push(mc, g0, gn)
                    if ig == 5 and mc + 1 < N_MC:
                        # Q for the next chunk; staged in out_ps so the score
                        # ring's double-buffer parity is undisturbed.
                        emit_proj(mc + 1, 0, pool=out_ps, tag="out")
            while state["q"]:
                drain_one()
            pmc, otb, rr = state["tail_pending"]
            emit_output_rt(pmc, otb, rr)
    nc.finalize()
    return nc


def _get_nc():
    global _NC_CACHE
    if _NC_CACHE is None:
        _NC_CACHE = build_nc()
    return _NC_CACHE


def make_in_maps(x, Wq, bq, Wk, bk, Wv, bv, Wo, bo):
    xT = np.ascontiguousarray(np.asarray(x, dtype=np.float32).T).astype(NP_MM_DT)
    maps = []
    for h in range(H):
        s = slice(h * DK, (h + 1) * DK)
        wqT = np.asarray(Wq, np.float32)[s, :].T  # [512, 64]
        wkT = np.asarray(Wk, np.float32)[s, :].T
        wvT = np.asarray(Wv, np.float32)[s, :].T
        w3 = np.concatenate([wqT, wqT, wkT, wkT, wvT], axis=1)  # [512, 320]
        b3 = np.zeros((128, 2), np.float32)
        b3[:, 0] = np.tile(np.asarray(bq, np.float32)[s], 2)
        b3[:, 1] = np.tile(np.asarray(bk, np.float32)[s], 2)
        maps.append(
            {
                "xt": xT,
                "w3": np.ascontiguousarray(w3).astype(NP_MM_DT),
                "b3": b3,
                "wot": np.ascontiguousarray(np.asarray(Wo, np.float32)[:, s].T).astype(NP_MM_DT),
            }
        )
    return maps


def _ensure_ntff_hook_shim():
    # The image's antenv package lacks axon_hooks; bass_utils imports it when
    # tracing is requested (including via the BASS_TRACE env var). Register a
    # ctypes-backed shim so that path works regardless of environment.
    if "antenv.axon_hooks" in sys.modules:
        return
    try:
        import contextlib
        import ctypes
        import types

        mod = types.ModuleType("antenv.axon_hooks")
        _state = {"hook": None}

        def set_axon_ntff_profile_hook(hook):
            _state["hook"] = hook

        def get_axon_ntff_profile_hook():
            if _state["hook"] is None:
                try:
                    lib = ctypes.CDLL("/opt/axon/libaxon_pjrt.so")
                except OSError:
                    return None
                if not hasattr(lib, "axon_start_nrt_profile"):
                    return None
                lib.axon_start_nrt_profile.argtypes = [ctypes.POINTER(ctypes.c_int64), ctypes.c_size_t]
                lib.axon_start_nrt_profile.restype = ctypes.c_int64
                lib.axon_stop_nrt_profile.argtypes = [ctypes.c_char_p]
                lib.axon_stop_nrt_profile.restype = ctypes.c_int64

                @contextlib.contextmanager
                def _hook(output_dir, device_ids):
                    import jax

                    jax.devices()
                    if device_ids:
                        ids = (ctypes.c_int64 * len(device_ids))(*device_ids)
                        rc = lib.axon_start_nrt_profile(ids, len(device_ids))
                    else:
                        rc = lib.axon_start_nrt_profile(None, 0)
                    if rc != 0:
                        raise RuntimeError(f"axon_start_nrt_profile rc={rc}")
                    try:
                        yield
                    finally:
                        n = lib.axon_stop_nrt_profile(str(output_dir).encode())
                        print(f"profile: {n} file(s) written to {output_dir}", file=sys.stderr)

                _state["hook"] = _hook
            return _state["hook"]

        mod.set_axon_ntff_profile_hook = set_axon_ntff_profile_hook
        mod.get_axon_ntff_profile_hook = get_axon_ntff_profile_hook
        sys.modules["antenv.axon_hooks"] = mod
        try:
            import antenv

            antenv.axon_hooks = mod
        except ImportError:
            pass
    except Exception:
        pass


def run(inputs, trace=False, **kw):
    _ensure_ntff_hook_shim()
    from concourse import bass_utils as BU
    from concourse.bass_utils import run_bass_kernel_spmd

    if not getattr(BU.upload_artifacts, "_safe", False):
        _orig_upload = BU.upload_artifacts

        def _safe_upload(tmpdir):
            try:
                return _orig_upload(tmpdir)
            except Exception:
                return f"local:{tmpdir}"

        _safe_upload._safe = True
        BU.upload_artifacts = _safe_upload

    nc = _get_nc()
    in_maps = make_in_maps(**inputs)
    res = run_bass_kernel_spmd(nc, in_maps, list(range(H)), trace=trace, **kw)
    bo = np.asarray(inputs["bo"], np.float32)
    # V-bias contribution: softmax rows sum to 1, so attn = W@(x Wv^T) + 1*bv^T
    # and y gains the constant row Wo @ bv (exact). Added here with bo.
    yv = np.asarray(inputs["Wo"], np.float32) @ np.asarray(inputs["bv"], np.float32)
    out = np.zeros((B, D), np.float32)
    for c in range(H):
        out += np.asarray(res.results[c]["y"], dtype=np.float32)
    out += (bo + yv)[None, :]
    return out, res


def kernel(**inputs):
    out, _ = run(inputs, trace=False)
    return out


# revision 56
# speedup vs baseline: 1.7010x; 1.0972x over previous
"""Multi-head cross-batch attention (B=4096, d_model=512, H=8 heads) on 8 TRN2 cores.

Sharding: one head per NeuronCore (tensor-parallel over H). Each core computes
its head's Q/K/V projections from a replicated (pre-transposed) x, the full
[4096, 4096] score block for that head, softmax (transposed layout, denominator
via a ones-column in V), attn @ V, and its partial out-projection
Y_h = attn_h @ Wo[:, h*64:(h+1)*64].T. Host sums the 8 partials and adds bo.

Layout notes (per core):
  - xT [512, 4096] (c on partitions) is fed from host so every matmul can
    contract over the partition dim without any on-device transpose of x.
  - QT/KT are stored duplicated across partition halves ([128, 4096]) so score
    matmuls can be row-packed two-at-a-time into the 128x128 PE array (the
    contraction dim is only 64).
  - V is projected directly token-major (x chunk as the stationary operand).
    The V bias is dropped on device: softmax rows sum to 1, so bv contributes
    the constant row Wo @ bv to y, which the host adds together with bo
    (exact). Similarly the K bias only shifts each softmax row by a constant
    and could be dropped, but it is kept since the add rides a drain copy.
  - Scores are computed transposed (ST[j, m]) so softmax's sum over keys j can
    ride the attn@V matmul: V is augmented with a ones column, making the
    accumulated output row 64 equal to sum_j exp(s). No max-subtraction is
    needed: scores are O(1) here (verified), so exp cannot overflow.
  - Normalization commutes with the out-projection: the r row is moved onto
    partitions per q-block by K=1 matmuls, inverted with fast 128-lane
    reciprocals, and applied on the post-projection drain. (A single-lane
    [1,512] reciprocal costs 3.35us on DVE and must never gate anything.)
  - Matmul inputs are bf16 (1 PE pass vs 2 for fp32, fast weight load);
    accumulation is fp32 in PSUM, exp inputs and the softmax denominator stay
    fp32. y partials return in bf16 (host accumulates in fp32).
  - A short warm-up matmul stream runs while x streams in, so the PE's HAM
    clock gate reaches 8/8 before the first projection matmul.
  - Emission is software-pipelined: attnV trails its scores/exp by one group so
    the PE's in-order queue never blocks on an in-flight exp; each chunk's
    output phase is emitted mid-way through the next chunk; chunk 0's score
    groups are interleaved into the projection loop as x arrives n-major;
    chunk n's Q projection is deferred into chunk n-1's group stream.
"""

import sys
from collections import deque

if "/opt/trn_rl_repo" not in sys.path:
    sys.path.insert(0, "/opt/trn_rl_repo")

import ml_dtypes
import numpy as np

import concourse.bass as bass
import concourse.tile as tile
from concourse import bacc, mybir

B = 4096
D = 512
H = 8
DK = 64
MC = 512  # query-chunk (m) width
N_MC = B // MC  # 8
JB = B // 128  # 32 j-blocks of 128 keys
F32 = mybir.dt.float32
BF16 = mybir.dt.bfloat16
MM_DT = BF16
NP_MM_DT = ml_dtypes.bfloat16 if MM_DT == BF16 else np.float32

# packed weight layout: per c-chunk of 128 channels, [wq_dup(128) | wk_dup(128) | wv(64)]
WCOLS = 128 + 128 + DK  # 320

# j-blocks per score/exp group: 3 blocks = 1536 floats = 3 PSUM banks.
# PSUM budget: 2x3 (score staging) + 1 (attnV accum) + 1 (vproj/outproj) = 8.
JGROUPS = [(0, 3), (3, 3), (6, 3), (9, 3), (12, 3), (15, 3), (18, 3), (21, 3), (24, 3), (27, 3), (30, 2)]

_NC_CACHE = None


def build_nc():
    nc = bacc.Bacc()

    xt = nc.dram_tensor("xt", [D, B], MM_DT, kind="ExternalInput")
    w3 = nc.dram_tensor("w3", [D, WCOLS], MM_DT, kind="ExternalInput")  # [c, wq|wk|wv]
    b3 = nc.dram_tensor("b3", [128, 2], F32, kind="ExternalInput")  # bq_dup | bk_dup
    wot = nc.dram_tensor("wot", [DK, D], MM_DT, kind="ExternalInput")
    y = nc.dram_tensor("y", [B, D], MM_DT, kind="ExternalOutput")

    with tile.TileContext(nc) as tc:
        with (
            tc.tile_pool(name="const", bufs=1) as const,
            tc.tile_pool(name="epool", bufs=20) as epool,
            tc.tile_pool(name="otpool", bufs=3) as otpool,
            tc.tile_pool(name="ypool", bufs=3) as ypool,
            tc.tile_pool(name="score_ps", bufs=2, space="PSUM") as score_ps,
            tc.tile_pool(name="attnv_ps", bufs=1, space="PSUM") as attnv_ps,
            tc.tile_pool(name="out_ps", bufs=1, space="PSUM") as out_ps,
        ):
            # ---- persistent SBUF ----
            x_sb = const.tile([128, 4 * B], MM_DT)  # 4 c-chunks side by side
            w3_sb = const.tile([128, 4 * WCOLS], MM_DT)  # 4 c-chunks of [128,320]
            b3_sb = const.tile([128, 2], F32)
            wot_sb = const.tile([DK, D], MM_DT)
            warm_sb = const.tile([128, MC], MM_DT)
            ones65 = const.tile([65, DK], MM_DT)  # row 64 = ones (K=1 broadcast matmul lhsT)
            qt_sb = const.tile([128, B], MM_DT)  # QT dup'd across partition halves
            kt_sb = const.tile([128, B], MM_DT)
            vp_sb = const.tile([128, JB * (DK + 1)], MM_DT)  # [V | 1] per j-block

            # ---- input DMAs ----
            # Co-queued DMAs on one ring complete nearly together, so the
            # weights get the scalar HWDGE ring to themselves (land first),
            # x chunk 0 gets the sync ring, and the rest of x spreads over all
            # three rings roughly in consumption order.
            x_sb3 = x_sb[:].rearrange("p (c n) -> p c n", c=4)
            xt3 = xt[:].rearrange("(c p) n -> p c n", p=128)
            nc.scalar.dma_start(
                out=w3_sb[:].rearrange("p (c n) -> p c n", c=4),
                in_=w3[:].rearrange("(c p) n -> p c n", p=128),
            )
            nc.scalar.dma_start(out=b3_sb[:], in_=b3[:])
            nc.sync.dma_start(out=x_sb3[:, :, 0:MC], in_=xt3[:, :, 0:MC])
            nc.sync.dma_start(out=wot_sb[:], in_=wot[:])
            x_dma_eng = {1: nc.gpsimd, 2: nc.sync, 3: nc.scalar, 4: nc.gpsimd, 5: nc.sync, 6: nc.scalar, 7: nc.gpsimd}
            for n in range(1, N_MC):
                x_dma_eng[n].dma_start(
                    out=x_sb3[:, :, n * MC : (n + 1) * MC],
                    in_=xt3[:, :, n * MC : (n + 1) * MC],
                )
            nc.vector.memset(warm_sb[:], 0.125)
            nc.vector.memset(ones65[DK : DK + 1, :], 1.0)
            # ones columns of the augmented-V layout
            nc.vector.memset(
                vp_sb[:].rearrange("p (t e) -> p t e", e=DK + 1)[:, :, DK : DK + 1], 1.0
            )
            # preload the exp activation table set before the first real exp
            # (junk output target so nothing downstream depends on it)
            junk_sb = const.tile([65, 4], F32)
            nc.scalar.activation(
                junk_sb[DK : DK + 1, :], ones65[DK : DK + 1, 0:4], mybir.ActivationFunctionType.Exp, scale=0.125
            )

            # ---- PE warm-up: long-stream matmuls (high busy duty) while x
            # streams in, so the HAM clock gate is at 8/8 when the first
            # projection matmul runs.
            for _ in range(13):
                wp = out_ps.tile([DK, MC], F32, tag="out")
                nc.tensor.matmul(wp[:], warm_sb[:, 0:DK], warm_sb[:], start=True, stop=True)

            # ---- emission helpers ----
            def emit_proj(n, off, pool=None, tag="score"):
                # one staging-ring unit holds one projection chunk [128, 512].
                # off 0 -> Q (w3 cols 0:128, bias col 0), off 1 -> K (cols 128:256, bias col 1)
                pp = (pool or score_ps).tile([128, MC], F32, tag=tag)
                for c in range(4):
                    nc.tensor.matmul(
                        pp[:],
                        w3_sb[:, c * WCOLS + off * 128 : c * WCOLS + off * 128 + 128],
                        x_sb[:, c * B + n * MC : c * B + (n + 1) * MC],
                        start=(c == 0),
                        stop=(c == 3),
                    )
                dst = qt_sb if off == 0 else kt_sb
                nc.vector.tensor_scalar(
                    out=dst[:, n * MC : (n + 1) * MC], in0=pp[:],
                    scalar1=b3_sb[:, off : off + 1], scalar2=None, op0=mybir.AluOpType.add,
                )

            def emit_v4(n):
                # V for j-blocks 4n..4n+3 token-major (x chunk stationary), no
                # bias (folded into a host-side constant), one strided drain.
                t0 = 4 * n
                vps = out_ps.tile([128, 4 * DK], F32, tag="out")
                for k in range(4):
                    dst = vps[:, k * DK : (k + 1) * DK]
                    for c in range(4):
                        nc.tensor.matmul(
                            dst,
                            x_sb[:, c * B + (t0 + k) * 128 : c * B + (t0 + k + 1) * 128],
                            w3_sb[:, c * WCOLS + 256 : c * WCOLS + 256 + DK],
                            start=(c == 0),
                            stop=(c == 3),
                        )
                nc.vector.tensor_copy(
                    vp_sb[:].rearrange("p (t e) -> p t e", e=DK + 1)[:, t0 : t0 + 4, 0:DK],
                    vps[:].rearrange("p (t e) -> p t e", e=DK),
                )

            def emit_sc(mc, g0, gn):
                m0 = mc * MC
                sp = score_ps.tile([128, gn * MC], F32, tag="score")
                et = epool.tile([128, gn * MC], MM_DT, tag="E")
                for k in range(gn):
                    jb = g0 + k
                    h0 = 64 * (jb % 2)
                    nc.tensor.matmul(
                        sp[:, k * MC : (k + 1) * MC],
                        kt_sb[h0 : h0 + 64, jb * 128 : (jb + 1) * 128],
                        qt_sb[h0 : h0 + 64, m0 : m0 + MC],
                        start=True,
                        stop=True,
                    )
                nc.scalar.activation(et[:], sp[:], mybir.ActivationFunctionType.Exp, scale=0.125)
                return et

            def emit_av(mc, g0, gn, et, av):
                for k in range(gn):
                    jb = g0 + k
                    nc.tensor.matmul(
                        av[:],
                        vp_sb[:, jb * (DK + 1) : (jb + 1) * (DK + 1)],
                        et[:, k * MC : (k + 1) * MC],
                        start=(jb == 0),
                        stop=(jb == JB - 1),
                    )

            def emit_otcopy(av):
                # Phase 1 of the output path: free av with FAST copies only
                # (single-lane reciprocal costs ~3.4us on DVE — measured — so
                # it must not hold av or the PE queue). The reciprocal runs
                # asynchronously on the copied r row; phase 2 is deferred
                # several drain slots to let it finish.
                otb = otpool.tile([DK, MC], MM_DT, tag="ot")
                nc.vector.tensor_copy(otb[:], av[0:DK, :])
                rr = otpool.tile([DK + 1, MC], MM_DT, tag="rrow")
                with nc.allow_low_precision(reason="softmax denominators are O(1); bf16 r costs ~0.2% rel"):
                    nc.vector.tensor_copy(rr[DK : DK + 1, :], av[DK : DK + 1, :])
                return rr, otb

            def emit_output_rt(mc, otb, rr):
                # Output path: per-q-block K=1 transpose of the r row +
                # 128-lane [128,1] reciprocals (~50ns each), scale after the
                # out-projection. All dependencies are PE-local or fast, so
                # no engine ever waits on a slow single-lane row reciprocal.
                m0 = mc * MC
                ysb = ypool.tile([128, 4 * MC], MM_DT, tag="y")
                for q in range(4):
                    rt = out_ps.tile([128, MC], F32, tag="out")
                    nc.tensor.matmul(
                        rt[:, 0:1], rr[DK : DK + 1, q * 128 : (q + 1) * 128],
                        ones65[DK : DK + 1, 0:1], start=True, stop=True,
                    )
                    rv = otpool.tile([128, 1], F32, tag="rv")
                    nc.vector.reciprocal(rv[:], rt[:, 0:1])
                    yp = out_ps.tile([128, MC], F32, tag="out")
                    nc.tensor.matmul(yp[:], otb[:, q * 128 : (q + 1) * 128], wot_sb[:], start=True, stop=True)
                    nc.vector.tensor_scalar(
                        out=ysb[:, q * MC : (q + 1) * MC], in0=yp[:],
                        scalar1=rv[:], scalar2=None, op0=mybir.AluOpType.mult,
                    )
                    if q == 1:
                        nc.gpsimd.dma_start(
                            out=y[m0 : m0 + 256, :].rearrange("(q p) d -> p q d", p=128),
                            in_=ysb[:, 0 : 2 * MC].rearrange("p (q d) -> p q d", q=2),
                        )
                nc.sync.dma_start(
                    out=y[m0 + 256 : m0 + MC, :].rearrange("(q p) d -> p q d", p=128),
                    in_=ysb[:, 2 * MC : 4 * MC].rearrange("p (q d) -> p q d", q=2),
                )

            # ---- software-pipelined main emission ----
            # attnV for a group is emitted one group behind its scores/exp, so
            # the PE's in-order queue never blocks on an in-flight exp. Each
            # chunk's output phase is emitted mid-way through the next chunk.
            # Chunk n's Q projection is emitted mid-way through chunk n-1.
            state = {"q": deque(), "av": None, "out_pending": None, "vn": 0}

            def drain_one():
                mc, g0, gn, et = state["q"].popleft()
                # just-in-time V projection: blocks g0..g0+gn-1 (+1 chunk ahead)
                while state["vn"] < N_MC and state["vn"] <= (g0 + gn - 1) // 4 + 1:
                    emit_v4(state["vn"])
                    state["vn"] += 1
                if state["av"] is None:
                    state["av"] = attnv_ps.tile([DK + 1, MC], F32, tag="attnv", name="av")
                emit_av(mc, g0, gn, et, state["av"])
                if g0 + gn == JB:  # chunk complete: free av (fast DVE copies)
                    rr, otb = emit_otcopy(state["av"])
                    state["av"] = None
                    state["out_pending"] = (mc, otb, rr)
                elif g0 >= 12 and state["out_pending"] is not None and state["out_pending"][0] == mc - 1:
                    emit_output_rt(*state["out_pending"])
                    state["out_pending"] = None

            def push(mc, g0, gn):
                # attnV trails scores/exp by TRAIL(mc) groups: deep for chunk 0
                # (spills chunk-0 attnV into later chunks' PE slack while exp
                # tracks the incoming x stream), shallow in steady state.
                et = emit_sc(mc, g0, gn)
                state["q"].append((mc, g0, gn, et))
                trail = {0: 6, 1: 4, 2: 3}.get(mc, 2)
                while len(state["q"]) > trail:
                    drain_one()

            # projections interleaved with chunk 0 (x arrives n-major).
            # K and V are needed globally and are emitted as x arrives; Q is
            # needed per-chunk: Q0/Q1 up front, Qn during chunk n-1.
            gi = 0
            for n in range(N_MC):
                emit_proj(n, 1)  # K
                if n <= 1:
                    emit_proj(n, 0)  # Q0, Q1
                while gi < len(JGROUPS) and JGROUPS[gi][0] + JGROUPS[gi][1] <= 4 * n + 4:
                    push(0, *JGROUPS[gi])
                    gi += 1
            while gi < len(JGROUPS):
                push(0, *JGROUPS[gi])
                gi += 1
            for mc in range(1, N_MC):
                for ig, (g0, gn) in enumerate(JGROUPS):
                    push(mc, g0, gn)
                    if ig == 5 and mc + 1 < N_MC:
                        # Q for the next chunk; staged in out_ps so the score
                        # ring's double-buffer parity is undisturbed.
                        emit_proj(mc + 1, 0, pool=out_ps, tag="out")
            while state["q"]:
                drain_one()
            emit_output_rt(*state["out_pending"])
    nc.finalize()
    return nc


def _get_nc():
    global _NC_CACHE
    if _NC_CACHE is None:
        _NC_CACHE = build_nc()
    return _NC_CACHE


def make_in_maps(x, Wq, bq, Wk, bk, Wv, bv, Wo, bo):
    xT = np.ascontiguousarray(np.asarray(x, dtype=np.float32).T).astype(NP_MM_DT)
    maps = []
    for h in range(H):
        s = slice(h * DK, (h + 1) * DK)
        wqT = np.asarray(Wq, np.float32)[s, :].T  # [512, 64]
        wkT = np.asarray(Wk, np.float32)[s, :].T
        wvT = np.asarray(Wv, np.float32)[s, :].T
        w3 = np.concatenate([wqT, wqT, wkT, wkT, wvT], axis=1)  # [512, 320]
        b3 = np.zeros((128, 2), np.float32)
        b3[:, 0] = np.tile(np.asarray(bq, np.float32)[s], 2)
        b3[:, 1] = np.tile(np.asarray(bk, np.float32)[s], 2)
        maps.append(
            {
                "xt": xT,
                "w3": np.ascontiguousarray(w3).astype(NP_MM_DT),
                "b3": b3,
                "wot": np.ascontiguousarray(np.asarray(Wo, np.float32)[:, s].T).astype(NP_MM_DT),
            }
        )
    return maps


def _ensure_ntff_hook_shim():
    # The image's antenv package lacks axon_hooks; bass_utils imports it when
    # tracing is requested (including via the BASS_TRACE env var). Register a
    # ctypes-backed shim so that path works regardless of environment.
    if "antenv.axon_hooks" in sys.modules:
        return
    try:
        import contextlib
        import ctypes
        import types

        mod = types.ModuleType("antenv.axon_hooks")
        _state = {"hook": None}

        def set_axon_ntff_profile_hook(hook):
            _state["hook"] = hook

        def get_axon_ntff_profile_hook():
            if _state["hook"] is None:
                try:
                    lib = ctypes.CDLL("/opt/axon/libaxon_pjrt.so")
                except OSError:
                    return None
                if not hasattr(lib, "axon_start_nrt_profile"):
                    return None
                lib.axon_start_nrt_profile.argtypes = [ctypes.POINTER(ctypes.c_int64), ctypes.c_size_t]
                lib.axon_start_nrt_profile.restype = ctypes.c_int64
                lib.axon_stop_nrt_profile.argtypes = [ctypes.c_char_p]
                lib.axon_stop_nrt_profile.restype = ctypes.c_int64

                @contextlib.contextmanager
                def _hook(output_dir, device_ids):
                    import jax

                    jax.devices()
                    if device_ids:
                        ids = (ctypes.c_int64 * len(device_ids))(*device_ids)
                        rc = lib.axon_start_nrt_profile(ids, len(device_ids))
                    else:
                        rc = lib.axon_start_nrt_profile(None, 0)
                    if rc != 0:
                        raise RuntimeError(f"axon_start_nrt_profile rc={rc}")
                    try:
                        yield
                    finally:
                        n = lib.axon_stop_nrt_profile(str(output_dir).encode())
                        print(f"profile: {n} file(s) written to {output_dir}", file=sys.stderr)

                _state["hook"] = _hook
            return _state["hook"]

        mod.set_axon_ntff_profile_hook = set_axon_ntff_profile_hook
        mod.get_axon_ntff_profile_hook = get_axon_ntff_profile_hook
        sys.modules["antenv.axon_hooks"] = mod
        try:
            import antenv

            antenv.axon_hooks = mod
        except ImportError:
            pass
    except Exception:
        pass


def run(inputs, trace=False, **kw):
    _ensure_ntff_hook_shim()
    from concourse import bass_utils as BU
    from concourse.bass_utils import run_bass_kernel_spmd

    if not getattr(BU.upload_artifacts, "_safe", False):
        _orig_upload = BU.upload_artifacts

        def _safe_upload(tmpdir):
            try:
                return _orig_upload(tmpdir)
            except Exception:
                return f"local:{tmpdir}"

        _safe_upload._safe = True
        BU.upload_artifacts = _safe_upload

    nc = _get_nc()
    in_maps = make_in_maps(**inputs)
    res = run_bass_kernel_spmd(nc, in_maps, list(range(H)), trace=trace, **kw)
    bo = np.asarray(inputs["bo"], np.float32)
    # V-bias contribution: softmax rows sum to 1, so attn = W@(x Wv^T) + 1*bv^T
    # and y gains the constant row Wo @ bv (exact). Added here with bo.
    yv = np.asarray(inputs["Wo"], np.float32) @ np.asarray(inputs["bv"], np.float32)
    out = np.zeros((B, D), np.float32)
    for c in range(H):
        out += np.asarray(res.results[c]["y"], dtype=np.float32)
    out += (bo + yv)[None, :]
    return out, res


def kernel(**inputs):
    out, _ = run(inputs, trace=False)
    return out


# revision 58
# speedup vs baseline: 1.7195x; 1.0108x over previous
"""Multi-head cross-batch attention (B=4096, d_model=512, H=8 heads) on 8 TRN2 cores.

Sharding: one head per NeuronCore (tensor-parallel over H). Each core computes
its head's Q/K/V projections from a replicated (pre-transposed) x, the full
[4096, 4096] score block for that head, softmax (transposed layout, denominator
via a ones-column in V), attn @ V, and its partial out-projection
Y_h = attn_h @ Wo[:, h*64:(h+1)*64].T. Host sums the 8 partials and adds bo.

Layout notes (per core):
  - xT [512, 4096] (c on partitions) is fed from host so every matmul can
    contract over the partition dim without any on-device transpose of x.
  - QT/KT are stored duplicated across partition halves ([128, 4096]) so score
    matmuls can be row-packed two-at-a-time into the 128x128 PE array (the
    contraction dim is only 64).
  - V is projected directly token-major (x chunk as the stationary operand).
    The V bias is dropped on device: softmax rows sum to 1, so bv contributes
    the constant row Wo @ bv to y, which the host adds together with bo
    (exact). Similarly the K bias only shifts each softmax row by a constant
    and could be dropped, but it is kept since the add rides a drain copy.
  - Scores are computed transposed (ST[j, m]) so softmax's sum over keys j can
    ride the attn@V matmul: V is augmented with a ones column, making the
    accumulated output row 64 equal to sum_j exp(s). No max-subtraction is
    needed: scores are O(1) here (verified), so exp cannot overflow.
  - Normalization commutes with the out-projection: the r row is moved onto
    partitions per q-block by K=1 matmuls, inverted with fast 128-lane
    reciprocals, and applied on the post-projection drain. (A single-lane
    [1,512] reciprocal costs 3.35us on DVE and must never gate anything.)
  - Matmul inputs are bf16 (1 PE pass vs 2 for fp32, fast weight load);
    accumulation is fp32 in PSUM, exp inputs and the softmax denominator stay
    fp32. y partials return in bf16 (host accumulates in fp32).
  - A short warm-up matmul stream runs while x streams in, so the PE's HAM
    clock gate reaches 8/8 before the first projection matmul.
  - Emission is software-pipelined: attnV trails its scores/exp by one group so
    the PE's in-order queue never blocks on an in-flight exp; each chunk's
    output phase is emitted mid-way through the next chunk; chunk 0's score
    groups are interleaved into the projection loop as x arrives n-major;
    chunk n's Q projection is deferred into chunk n-1's group stream.
"""

import sys
from collections import deque

if "/opt/trn_rl_repo" not in sys.path:
    sys.path.insert(0, "/opt/trn_rl_repo")

import ml_dtypes
import numpy as np

import concourse.bass as bass
import concourse.tile as tile
from concourse import bacc, mybir

B = 4096
D = 512
H = 8
DK = 64
MC = 512  # query-chunk (m) width
N_MC = B // MC  # 8
JB = B // 128  # 32 j-blocks of 128 keys
F32 = mybir.dt.float32
BF16 = mybir.dt.bfloat16
MM_DT = BF16
NP_MM_DT = ml_dtypes.bfloat16 if MM_DT == BF16 else np.float32
# x and the packed projection weights travel as fp8e4m3 (halves the x DMA,
# which gates chunk 0). Weights are scaled by WSCALE to clear the fp8
# subnormal range; Q,K pick up WSCALE each (undone by the exp scale), V picks
# up WSCALE (cancelled by setting the softmax ones-column to WSCALE so the
# numerator and denominator scale together).
XW_DT = mybir.dt.float8e4
NP_XW_DT = ml_dtypes.float8_e4m3
WSCALE = 32.0

# packed weight layout: per c-chunk of 128 channels, [wq_dup(128) | wk_dup(128) | wv(64)]
WCOLS = 128 + 128 + DK  # 320

# j-blocks per score/exp group: 3 blocks = 1536 floats = 3 PSUM banks.
# PSUM budget: 2x3 (score staging) + 1 (attnV accum) + 1 (vproj/outproj) = 8.
JGROUPS = [(0, 3), (3, 3), (6, 3), (9, 3), (12, 3), (15, 3), (18, 3), (21, 3), (24, 3), (27, 3), (30, 2)]

_NC_CACHE = None


def build_nc():
    nc = bacc.Bacc()

    xt = nc.dram_tensor("xt", [D, B], XW_DT, kind="ExternalInput")
    w3 = nc.dram_tensor("w3", [D, WCOLS], XW_DT, kind="ExternalInput")  # [c, wq|wk|wv]
    b3 = nc.dram_tensor("b3", [128, 2], F32, kind="ExternalInput")  # bq_dup | bk_dup
    wot = nc.dram_tensor("wot", [DK, D], MM_DT, kind="ExternalInput")
    y = nc.dram_tensor("y", [B, D], MM_DT, kind="ExternalOutput")

    with tile.TileContext(nc) as tc:
        with (
            tc.tile_pool(name="const", bufs=1) as const,
            tc.tile_pool(name="epool", bufs=20) as epool,
            tc.tile_pool(name="otpool", bufs=3) as otpool,
            tc.tile_pool(name="ypool", bufs=3) as ypool,
            tc.tile_pool(name="score_ps", bufs=2, space="PSUM") as score_ps,
            tc.tile_pool(name="attnv_ps", bufs=1, space="PSUM") as attnv_ps,
            tc.tile_pool(name="out_ps", bufs=1, space="PSUM") as out_ps,
        ):
            # ---- persistent SBUF ----
            x_sb = const.tile([128, 4 * B], XW_DT)  # 4 c-chunks side by side
            w3_sb = const.tile([128, 4 * WCOLS], XW_DT)  # 4 c-chunks of [128,320]
            b3_sb = const.tile([128, 2], F32)
            wot_sb = const.tile([DK, D], MM_DT)
            warm_sb = const.tile([128, MC], MM_DT)
            ones65 = const.tile([65, DK], MM_DT)  # row 64 = ones (K=1 broadcast matmul lhsT)
            qt_sb = const.tile([128, B], MM_DT)  # QT dup'd across partition halves
            kt_sb = const.tile([128, B], MM_DT)
            vp_sb = const.tile([128, JB * (DK + 1)], MM_DT)  # [V | 1] per j-block

            # ---- input DMAs ----
            # Co-queued DMAs on one ring complete nearly together, so the
            # weights get the scalar HWDGE ring to themselves (land first),
            # x chunk 0 gets the sync ring, and the rest of x spreads over all
            # three rings roughly in consumption order.
            x_sb3 = x_sb[:].rearrange("p (c n) -> p c n", c=4)
            xt3 = xt[:].rearrange("(c p) n -> p c n", p=128)
            nc.scalar.dma_start(
                out=w3_sb[:].rearrange("p (c n) -> p c n", c=4),
                in_=w3[:].rearrange("(c p) n -> p c n", p=128),
            )
            nc.scalar.dma_start(out=b3_sb[:], in_=b3[:])
            nc.sync.dma_start(out=x_sb3[:, :, 0:MC], in_=xt3[:, :, 0:MC])
            nc.sync.dma_start(out=wot_sb[:], in_=wot[:])
            x_dma_eng = {1: nc.gpsimd, 2: nc.sync, 3: nc.scalar, 4: nc.gpsimd, 5: nc.sync, 6: nc.scalar, 7: nc.gpsimd}
            for n in range(1, N_MC):
                x_dma_eng[n].dma_start(
                    out=x_sb3[:, :, n * MC : (n + 1) * MC],
                    in_=xt3[:, :, n * MC : (n + 1) * MC],
                )
            nc.vector.memset(warm_sb[:], 0.125)
            nc.vector.memset(ones65[DK : DK + 1, :], 1.0)
            # ones columns of the augmented-V layout
            nc.vector.memset(
                vp_sb[:].rearrange("p (t e) -> p t e", e=DK + 1)[:, :, DK : DK + 1], WSCALE
            )
            # preload the exp activation table set before the first real exp
            # (junk output target so nothing downstream depends on it)
            junk_sb = const.tile([65, 4], F32)
            nc.scalar.activation(
                junk_sb[DK : DK + 1, :], ones65[DK : DK + 1, 0:4], mybir.ActivationFunctionType.Exp, scale=0.125
            )

            # ---- PE warm-up: long-stream matmuls (high busy duty) while x
            # streams in, so the HAM clock gate is at 8/8 when the first
            # projection matmul runs.
            for _ in range(13):
                wp = out_ps.tile([DK, MC], F32, tag="out")
                nc.tensor.matmul(wp[:], warm_sb[:, 0:DK], warm_sb[:], start=True, stop=True)

            # ---- emission helpers ----
            def emit_proj(n, off, pool=None, tag="score"):
                # one staging-ring unit holds one projection chunk [128, 512].
                # off 0 -> Q (w3 cols 0:128, bias col 0), off 1 -> K (cols 128:256, bias col 1)
                pp = (pool or score_ps).tile([128, MC], F32, tag=tag)
                for c in range(4):
                    nc.tensor.matmul(
                        pp[:],
                        w3_sb[:, c * WCOLS + off * 128 : c * WCOLS + off * 128 + 128],
                        x_sb[:, c * B + n * MC : c * B + (n + 1) * MC],
                        start=(c == 0),
                        stop=(c == 3),
                    )
                dst = qt_sb if off == 0 else kt_sb
                nc.vector.tensor_scalar(
                    out=dst[:, n * MC : (n + 1) * MC], in0=pp[:],
                    scalar1=b3_sb[:, off : off + 1], scalar2=None, op0=mybir.AluOpType.add,
                )

            def emit_v4(n):
                # V for j-blocks 4n..4n+3 token-major (x chunk stationary), no
                # bias (folded into a host-side constant), one strided drain.
                t0 = 4 * n
                vps = out_ps.tile([128, 4 * DK], F32, tag="out")
                for k in range(4):
                    dst = vps[:, k * DK : (k + 1) * DK]
                    for c in range(4):
                        nc.tensor.matmul(
                            dst,
                            x_sb[:, c * B + (t0 + k) * 128 : c * B + (t0 + k + 1) * 128],
                            w3_sb[:, c * WCOLS + 256 : c * WCOLS + 256 + DK],
                            start=(c == 0),
                            stop=(c == 3),
                        )
                nc.vector.tensor_copy(
                    vp_sb[:].rearrange("p (t e) -> p t e", e=DK + 1)[:, t0 : t0 + 4, 0:DK],
                    vps[:].rearrange("p (t e) -> p t e", e=DK),
                )

            def emit_sc(mc, g0, gn):
                m0 = mc * MC
                sp = score_ps.tile([128, gn * MC], F32, tag="score")
                et = epool.tile([128, gn * MC], MM_DT, tag="E")
                for k in range(gn):
                    jb = g0 + k
                    h0 = 64 * (jb % 2)
                    nc.tensor.matmul(
                        sp[:, k * MC : (k + 1) * MC],
                        kt_sb[h0 : h0 + 64, jb * 128 : (jb + 1) * 128],
                        qt_sb[h0 : h0 + 64, m0 : m0 + MC],
                        start=True,
                        stop=True,
                    )
                nc.scalar.activation(et[:], sp[:], mybir.ActivationFunctionType.Exp, scale=0.125 / (WSCALE * WSCALE))
                return et

            def emit_av(mc, g0, gn, et, av):
                for k in range(gn):
                    jb = g0 + k
                    nc.tensor.matmul(
                        av[:],
                        vp_sb[:, jb * (DK + 1) : (jb + 1) * (DK + 1)],
                        et[:, k * MC : (k + 1) * MC],
                        start=(jb == 0),
                        stop=(jb == JB - 1),
                    )

            def emit_otcopy(av):
                # Phase 1 of the output path: free av with FAST copies only
                # (single-lane reciprocal costs ~3.4us on DVE — measured — so
                # it must not hold av or the PE queue). The reciprocal runs
                # asynchronously on the copied r row; phase 2 is deferred
                # several drain slots to let it finish.
                otb = otpool.tile([DK, MC], MM_DT, tag="ot")
                nc.vector.tensor_copy(otb[:], av[0:DK, :])
                rr = otpool.tile([DK + 1, MC], MM_DT, tag="rrow")
                with nc.allow_low_precision(reason="softmax denominators are O(1); bf16 r costs ~0.2% rel"):
                    nc.vector.tensor_copy(rr[DK : DK + 1, :], av[DK : DK + 1, :])
                return rr, otb

            def emit_output_rt(mc, otb, rr, pool=None, tag="out"):
                # Output path: per-q-block K=1 transpose of the r row +
                # 128-lane [128,1] reciprocals (~50ns each), scale after the
                # out-projection. All dependencies are PE-local or fast, so
                # no engine ever waits on a slow single-lane row reciprocal.
                m0 = mc * MC
                ysb = ypool.tile([128, 4 * MC], MM_DT, tag="y")
                for q in range(4):
                    rt = (pool or out_ps).tile([128, MC], F32, tag=tag)
                    nc.tensor.matmul(
                        rt[:, 0:1], rr[DK : DK + 1, q * 128 : (q + 1) * 128],
                        ones65[DK : DK + 1, 0:1], start=True, stop=True,
                    )
                    rv = otpool.tile([128, 1], F32, tag="rv")
                    nc.vector.reciprocal(rv[:], rt[:, 0:1])
                    yp = (pool or out_ps).tile([128, MC], F32, tag=tag)
                    nc.tensor.matmul(yp[:], otb[:, q * 128 : (q + 1) * 128], wot_sb[:], start=True, stop=True)
                    nc.vector.tensor_scalar(
                        out=ysb[:, q * MC : (q + 1) * MC], in0=yp[:],
                        scalar1=rv[:], scalar2=None, op0=mybir.AluOpType.mult,
                    )
                    if q == 1:
                        nc.gpsimd.dma_start(
                            out=y[m0 : m0 + 256, :].rearrange("(q p) d -> p q d", p=128),
                            in_=ysb[:, 0 : 2 * MC].rearrange("p (q d) -> p q d", q=2),
                        )
                nc.sync.dma_start(
                    out=y[m0 + 256 : m0 + MC, :].rearrange("(q p) d -> p q d", p=128),
                    in_=ysb[:, 2 * MC : 4 * MC].rearrange("p (q d) -> p q d", q=2),
                )

            # ---- software-pipelined main emission ----
            # attnV for a group is emitted one group behind its scores/exp, so
            # the PE's in-order queue never blocks on an in-flight exp. Each
            # chunk's output phase is emitted mid-way through the next chunk.
            # Chunk n's Q projection is emitted mid-way through chunk n-1.
            state = {"q": deque(), "av": None, "out_pending": None, "vn": 0}

            def drain_one():
                mc, g0, gn, et = state["q"].popleft()
                # just-in-time V projection: blocks g0..g0+gn-1 (+1 chunk ahead)
                while state["vn"] < N_MC and state["vn"] <= (g0 + gn - 1) // 4 + 1:
                    emit_v4(state["vn"])
                    state["vn"] += 1
                if state["av"] is None:
                    state["av"] = attnv_ps.tile([DK + 1, MC], F32, tag="attnv", name="av")
                emit_av(mc, g0, gn, et, state["av"])
                if g0 + gn == JB:  # chunk complete: free av (fast DVE copies)
                    rr, otb = emit_otcopy(state["av"])
                    state["av"] = None
                    state["out_pending"] = (mc, otb, rr)
                elif g0 >= 12 and state["out_pending"] is not None and state["out_pending"][0] == mc - 1:
                    emit_output_rt(*state["out_pending"])
                    state["out_pending"] = None

            def push(mc, g0, gn):
                # attnV trails scores/exp by TRAIL(mc) groups: deep for chunk 0
                # (spills chunk-0 attnV into later chunks' PE slack while exp
                # tracks the incoming x stream), shallow in steady state.
                et = emit_sc(mc, g0, gn)
                state["q"].append((mc, g0, gn, et))
                trail = {0: 6, 1: 4, 2: 3}.get(mc, 2)
                while len(state["q"]) > trail:
                    drain_one()

            # projections interleaved with chunk 0 (x arrives n-major).
            # K and V are needed globally and are emitted as x arrives; Q is
            # needed per-chunk: Q0/Q1 up front, Qn during chunk n-1.
            gi = 0
            for n in range(N_MC):
                emit_proj(n, 1)  # K
                if n <= 1:
                    emit_proj(n, 0)  # Q0, Q1
                while gi < len(JGROUPS) and JGROUPS[gi][0] + JGROUPS[gi][1] <= 4 * n + 4:
                    push(0, *JGROUPS[gi])
                    gi += 1
            while gi < len(JGROUPS):
                push(0, *JGROUPS[gi])
                gi += 1
            for mc in range(1, N_MC):
                for ig, (g0, gn) in enumerate(JGROUPS):
                    push(mc, g0, gn)
                    if ig == 5 and mc + 1 < N_MC:
                        # Q for the next chunk; staged in out_ps so the score
                        # ring's double-buffer parity is undisturbed.
                        emit_proj(mc + 1, 0, pool=out_ps, tag="out")
            while state["q"]:
                drain_one()
            pmc, otb, rr = state["out_pending"]
            emit_output_rt(pmc, otb, rr, pool=score_ps, tag="score")
    nc.finalize()
    return nc


def _get_nc():
    global _NC_CACHE
    if _NC_CACHE is None:
        _NC_CACHE = build_nc()
    return _NC_CACHE


def make_in_maps(x, Wq, bq, Wk, bk, Wv, bv, Wo, bo):
    xT = np.ascontiguousarray(np.asarray(x, dtype=np.float32).T).astype(NP_XW_DT)
    maps = []
    for h in range(H):
        s = slice(h * DK, (h + 1) * DK)
        wqT = np.asarray(Wq, np.float32)[s, :].T  # [512, 64]
        wkT = np.asarray(Wk, np.float32)[s, :].T
        wvT = np.asarray(Wv, np.float32)[s, :].T
        w3 = WSCALE * np.concatenate([wqT, wqT, wkT, wkT, wvT], axis=1)  # [512, 320]
        b3 = np.zeros((128, 2), np.float32)
        b3[:, 0] = WSCALE * np.tile(np.asarray(bq, np.float32)[s], 2)
        b3[:, 1] = WSCALE * np.tile(np.asarray(bk, np.float32)[s], 2)
        maps.append(
            {
                "xt": xT,
                "w3": np.ascontiguousarray(w3).astype(NP_XW_DT),
                "b3": b3,
                "wot": np.ascontiguousarray(np.asarray(Wo, np.float32)[:, s].T).astype(NP_MM_DT),
            }
        )
    return maps


def _ensure_ntff_hook_shim():
    # The image's antenv package lacks axon_hooks; bass_utils imports it when
    # tracing is requested (including via the BASS_TRACE env var). Register a
    # ctypes-backed shim so that path works regardless of environment.
    if "antenv.axon_hooks" in sys.modules:
        return
    try:
        import contextlib
        import ctypes
        import types

        mod = types.ModuleType("antenv.axon_hooks")
        _state = {"hook": None}

        def set_axon_ntff_profile_hook(hook):
            _state["hook"] = hook

        def get_axon_ntff_profile_hook():
            if _state["hook"] is None:
                try:
                    lib = ctypes.CDLL("/opt/axon/libaxon_pjrt.so")
                except OSError:
                    return None
                if not hasattr(lib, "axon_start_nrt_profile"):
                    return None
                lib.axon_start_nrt_profile.argtypes = [ctypes.POINTER(ctypes.c_int64), ctypes.c_size_t]
                lib.axon_start_nrt_profile.restype = ctypes.c_int64
                lib.axon_stop_nrt_profile.argtypes = [ctypes.c_char_p]
                lib.axon_stop_nrt_profile.restype = ctypes.c_int64

                @contextlib.contextmanager
                def _hook(output_dir, device_ids):
                    import jax

                    jax.devices()
                    if device_ids:
                        ids = (ctypes.c_int64 * len(device_ids))(*device_ids)
                        rc = lib.axon_start_nrt_profile(ids, len(device_ids))
                    else:
                        rc = lib.axon_start_nrt_profile(None, 0)
                    if rc != 0:
                        raise RuntimeError(f"axon_start_nrt_profile rc={rc}")
                    try:
                        yield
                    finally:
                        n = lib.axon_stop_nrt_profile(str(output_dir).encode())
                        print(f"profile: {n} file(s) written to {output_dir}", file=sys.stderr)

                _state["hook"] = _hook
            return _state["hook"]

        mod.set_axon_ntff_profile_hook = set_axon_ntff_profile_hook
        mod.get_axon_ntff_profile_hook = get_axon_ntff_profile_hook
        sys.modules["antenv.axon_hooks"] = mod
        try:
            import antenv

            antenv.axon_hooks = mod
        except ImportError:
            pass
    except Exception:
        pass


def run(inputs, trace=False, **kw):
    _ensure_ntff_hook_shim()
    from concourse import bass_utils as BU
    from concourse.bass_utils import run_bass_kernel_spmd

    if not getattr(BU.upload_artifacts, "_safe", False):
        _orig_upload = BU.upload_artifacts

        def _safe_upload(tmpdir):
            try:
                return _orig_upload(tmpdir)
            except Exception:
                return f"local:{tmpdir}"

        _safe_upload._safe = True
        BU.upload_artifacts = _safe_upload

    nc = _get_nc()
    in_maps = make_in_maps(**inputs)
    res = run_bass_kernel_spmd(nc, in_maps, list(range(H)), trace=trace, **kw)
    bo = np.asarray(inputs["bo"], np.float32)
    # V-bias contribution: softmax rows sum to 1, so attn = W@(x Wv^T) + 1*bv^T
    # and y gains the constant row Wo @ bv (exact). Added here with bo.
    yv = np.asarray(inputs["Wo"], np.float32) @ np.asarray(inputs["bv"], np.float32)
    out = np.zeros((B, D), np.float32)
    for c in range(H):
        out += np.asarray(res.results[c]["y"], dtype=np.float32)
    out += (bo + yv)[None, :]
    return out, res


def kernel(**inputs):
    out, _ = run(inputs, trace=False)
    return out
